# revision 1
# baseline (speedup 1.0000x reference)
"""Self-contained Trainium2 Bass kernel for nn_DariushLayer_14087492731059.

kernel(**inputs) takes the FULL unsharded inputs of reference.setup_inputs()
and returns the full [B, S, D] float32 output, computed across 8 NeuronCores:
attention tensor-parallel over heads (2 heads/core), MoE expert-parallel
(1 expert/core), one SPMD launch with an on-device AllReduce at the
attention->MoE boundary.  GEMMs run in fp32r (12-bit-mantissa fp32, 4x rate).
"""

import numpy as np
import concourse.bass as bass
import concourse.tile as tile
from concourse import bacc, mybir
from contextlib import ExitStack

f32, f32r = mybir.dt.float32, mybir.dt.float32r
AF = mybir.ActivationFunctionType
OP = mybir.AluOpType
AX = mybir.AxisListType

B, S, D, H, DK, E = 2, 2048, 1024, 16, 64, 8
T = B * S
NC = 8
KC = D // 128
EPS = 1e-6
MASKNEG = -30000.0


def build_program():
    nc = bacc.Bacc("TRN2", target_bir_lowering=False, debug=False, num_devices=NC)
    dt = nc.dram_tensor
    io = {}
    def inp(nm, shp, ty=f32):
        io[nm] = dt(nm, shp, ty, kind="ExternalInput").ap()
    def outp(nm, shp, ty=f32):
        io[nm] = dt(nm, shp, ty, kind="ExternalOutput").ap()
    inp("x", [T, D])
    for nm in ("wq", "wk", "wv", "wqs", "wks"):
        inp(nm, [D, 128], f32r)
    inp("wo", [128, D], f32r)
    inp("cosb", [128, S]); inp("sinb", [128, S])
    inp("masks", [128, 4, 512])
    inp("ident", [128, 128]); inp("id64", [128, 128])
    inp("rw", [D, E], f32r)
    inp("noise", [T, E])
    inp("sel", [128, E])
    inp("w1", [KC, D, 128], f32r); inp("w2", [KC, D, 128], f32r)
    inp("wout", [D, D], f32r)
    inp("b1h", [128, KC]); inp("b2h", [128, KC])
    outp("out", [T, D])
    outp("h_out", [T, D])

    with tile.TileContext(nc) as tc, ExitStack() as top:
        const = top.enter_context(tc.tile_pool(name="const", bufs=1))
        psum = top.enter_context(tc.tile_pool(name="psum", bufs=8, space="PSUM"))
        dram = top.enter_context(tc.tile_pool(name="dram", bufs=1, space="DRAM"))

        def P(shape=(128, 512)):
            return psum.tile(list(shape), f32, tag="ps", name="ps")

        cst = {}
        for nm, shp in [("ident", [128, 128]), ("id64", [128, 128]),
                        ("sel", [128, E]), ("b1h", [128, KC]), ("b2h", [128, KC])]:
            cst[nm] = const.tile(shp, f32, name=nm)
            nc.sync.dma_start(cst[nm][:], io[nm][:])
        cst["rw"] = const.tile([128, KC, E], f32r, name="rw")
        nc.sync.dma_start(cst["rw"][:], io["rw"].rearrange("(kc p) m -> p kc m", p=128))
        ones = const.tile([128, 1], f32, name="ones")
        nc.vector.memset(ones[:], 1.0)
        eps_t = const.tile([128, 1], f32, name="eps_t")
        nc.vector.memset(eps_t[:], EPS)
        zeros_t = const.tile([128, 1], f32, name="zeros_t")
        nc.vector.memset(zeros_t[:], 0.0)
        wgt_all = const.tile([128, 32], f32, name="wgt_all")

        ar_in = dram.tile([T, D], f32, name="ar_in")
        ar_out = dram.tile([T, D], f32, name="ar_out", addr_space="Shared")

        # --- rmsnorm one [128, D] row-tile and transpose into xT[:, kc, lo] ---
        def norm_transpose(work, src_dram, xT, st, lo, extra=None, src2=None):
            xt = work.tile([128, D], f32, tag="xt", name="xt")
            r0 = st * 128
            nc.sync.dma_start(xt[:], src_dram[r0:r0 + 128, :])
            if src2 is not None:
                a2 = work.tile([128, D], f32, tag="a2", name="a2", bufs=1)
                nc.sync.dma_start(a2[:], src2[r0:r0 + 128, :])
                nc.vector.tensor_tensor(xt[:], xt[:], a2[:], op=OP.add)
            if extra is not None:
                extra(xt)
            sq = work.tile([128, D], f32, tag="xh", name="sq")
            nc.vector.tensor_tensor(sq[:], xt[:], xt[:], op=OP.mult)
            ssum = work.tile([128, 1], f32, tag="ssum", name="ssum")
            nc.vector.reduce_sum(ssum[:], sq[:], axis=AX.X)
            sd = work.tile([128, 1], f32, tag="ssum", name="sd")
            nc.scalar.activation(sd[:], ssum[:], AF.Sqrt, bias=eps_t[:], scale=1.0 / D)
            rr = work.tile([128, 1], f32, tag="ssum", name="rr")
            nc.vector.reciprocal(rr[:], sd[:])
            xh = work.tile([128, D], f32, tag="xh", name="xh")
            nc.vector.tensor_scalar_mul(xh[:], xt[:], rr[:])
            for kc in range(KC):
                pt = P((128, 128))
                nc.tensor.transpose(pt[:], xh[:, kc * 128:(kc + 1) * 128],
                                    cst["ident"][:])
                if kc % 2 == 0:
                    nc.scalar.copy(xT[:, kc, lo:lo + 128], pt[:])
                else:
                    nc.vector.tensor_copy(xT[:, kc, lo:lo + 128], pt[:])

        # =================================================================
        # Phase A: attention
        # =================================================================
        with tc.tile_pool(name="qkv", bufs=1) as qkv, \
             tc.tile_pool(name="apool", bufs=1) as apool:
            qT = qkv.tile([128, T], f32r, name="qT")
            kT = qkv.tile([128, T], f32r, name="kT")
            vT = qkv.tile([128, T], f32, name="vT")
            for nm, shp, ty in [("cosb", [128, S], f32), ("sinb", [128, S], f32),
                                ("masks", [128, 4, 512], f32)]:
                cst[nm] = apool.tile(shp, ty, name=nm)
                nc.sync.dma_start(cst[nm][:], io[nm][:])
            for nm in ("wq", "wk", "wv", "wqs", "wks"):
                cst[nm] = apool.tile([128, KC, 128], f32r, name=nm)
                nc.sync.dma_start(cst[nm][:],
                                  io[nm].rearrange("(kc p) m -> p kc m", p=128))

            with tc.tile_pool(name="xnt", bufs=2) as xnt_pool, \
                 tc.tile_pool(name="work", bufs=2) as work:
                for b in range(B):
                    for sb in range(4):
                        xnT = xnt_pool.tile([128, KC, 512], f32r, tag="xnT",
                                            name="xnT")
                        for q4 in range(4):
                            st = b * 16 + sb * 4 + q4
                            norm_transpose(work, io["x"], xnT, st, q4 * 128)
                        gl = slice(b * S + sb * 512, b * S + (sb + 1) * 512)
                        sl = slice(sb * 512, (sb + 1) * 512)
                        for base, swp, dst in (("wq", "wqs", qT), ("wk", "wks", kT)):
                            pa = P()
                            for kc in range(KC):
                                nc.tensor.matmul(pa[:], cst[base][:, kc, :],
                                                 xnT[:, kc, :],
                                                 start=(kc == 0), stop=(kc == KC - 1))
                            pb = P()
                            for kc in range(KC):
                                nc.tensor.matmul(pb[:], cst[swp][:, kc, :],
                                                 xnT[:, kc, :],
                                                 start=(kc == 0), stop=(kc == KC - 1))
                            t1 = work.tile([128, 512], f32, tag="t1", name="t1")
                            nc.vector.tensor_tensor(t1[:], pa[:], cst["cosb"][:, sl],
                                                    op=OP.mult)
                            t2 = work.tile([128, 512], f32, tag="t2", name="t2")
                            nc.vector.tensor_tensor(t2[:], pb[:], cst["sinb"][:, sl],
                                                    op=OP.mult)
                            nc.vector.tensor_tensor(dst[:, gl], t1[:], t2[:], op=OP.add)
                        pv = P()
                        for kc in range(KC):
                            nc.tensor.matmul(pv[:], cst["wv"][:, kc, :], xnT[:, kc, :],
                                             start=(kc == 0), stop=(kc == KC - 1))
                        nc.scalar.copy(vT[:, gl], pv[:])

            # attention core
            with tc.tile_pool(name="attw", bufs=1) as attw, \
                 tc.tile_pool(name="att", bufs=3) as att, \
                 tc.tile_pool(name="expp", bufs=4) as expp, \
                 tc.tile_pool(name="vsb", bufs=18) as vsbp:
                oT0 = attw.tile([64, T], f32r, name="oT0")
                oT1 = attw.tile([64, T], f32r, name="oT1")
                oTs = [oT0, oT1]
                for b in range(B):
                    for h in range(2):
                        hr = slice(h * 64, (h + 1) * 64)
                        idn = cst["ident"] if h == 0 else cst["id64"]
                        vchunks = []
                        for m in range(16):
                            gk = slice(b * S + m * 128, b * S + (m + 1) * 128)
                            pt = P((128, 64))
                            nc.tensor.transpose(pt[:], vT[hr, gk], idn[hr, 0:64])
                            vs = vsbp.tile([128, 66], f32r, tag="vs", name="vs")
                            nc.scalar.copy(vs[:, 0:64], pt[:])
                            nc.vector.tensor_copy(vs[:, 64:65], ones[:])
                            nc.vector.tensor_copy(vs[:, 65:66], zeros_t[:])
                            vchunks.append(vs)
                        for jq in range(4):
                            gq = slice(b * S + jq * 512, b * S + (jq + 1) * 512)
                            nch = 4 * jq + 4
                            pos = [P((128, 66)) for _ in range(4)]
                            for m in range(nch):
                                gk = slice(b * S + m * 128, b * S + (m + 1) * 128)
                                pse = P()
                                nc.tensor.matmul(pse[:], kT[hr, gk], qT[hr, gq],
                                                 start=True, stop=True)
                                if m >= 4 * jq:
                                    nc.vector.tensor_tensor(
                                        pse[:], pse[:],
                                        cst["masks"][:, m - 4 * jq, :], op=OP.add)
                                et = expp.tile([128, 512], f32r, tag="et", name="et")
                                nc.scalar.activation(et[:], pse[:], AF.Exp, scale=0.125)
                                for qt in range(4):
                                    nc.tensor.matmul(
                                        pos[qt][:], et[:, qt * 128:(qt + 1) * 128],
                                        vchunks[m][:],
                                        start=(m == 0), stop=(m == nch - 1))
                            for qt in range(4):
                                rcp = att.tile([128, 1], f32, tag="rcp", name="rcp")
                                nc.vector.reciprocal(rcp[:], pos[qt][:, 64:65])
                                opr = att.tile([128, 64], f32, tag="opr", name="opr")
                                nc.vector.tensor_scalar_mul(opr[:], pos[qt][:, 0:64],
                                                            rcp[:])
                                ptt = P((128, 128))
                                nc.tensor.transpose(ptt[0:64, :], opr[:],
                                                    cst["ident"][:])
                                g128 = slice(b * S + jq * 512 + qt * 128,
                                             b * S + jq * 512 + (qt + 1) * 128)
                                nc.scalar.copy(oTs[h][:, g128], ptt[0:64, :])

                # out-projection partials -> ar_in
                wo0 = attw.tile([64, D], f32r, name="wo0")
                wo1 = attw.tile([64, D], f32r, name="wo1")
                nc.sync.dma_start(wo0[:], io["wo"][0:64, :])
                nc.sync.dma_start(wo1[:], io["wo"][64:128, :])
                for st in range(32):
                    g = slice(st * 128, (st + 1) * 128)
                    for db in range(2):
                        dsl = slice(db * 512, (db + 1) * 512)
                        pp = P()
                        nc.tensor.matmul(pp[:], oT0[:, g], wo0[:, dsl],
                                         start=True, stop=False)
                        nc.tensor.matmul(pp[:], oT1[:, g], wo1[:, dsl],
                                         start=False, stop=True)
                        ab = att.tile([128, 512], f32, tag="ab", name="ab")
                        nc.scalar.copy(ab[:], pp[:])
                        nc.sync.dma_start(ar_in[g, dsl], ab[:])

        nc.gpsimd.collective_compute(
            "AllReduce", OP.add, replica_groups=[list(range(NC))],
            ins=[ar_in.opt()], outs=[ar_out.opt()])

        # =================================================================
        # Phase B: MoE
        # =================================================================
        with tc.tile_pool(name="bpool", bufs=1) as bpool, \
             tc.tile_pool(name="hnt", bufs=2) as hnt_pool, \
             tc.tile_pool(name="moe", bufs=2) as moe, \
             tc.tile_pool(name="workb", bufs=2) as work:
            wout_t = bpool.tile([128, KC, D], f32r, name="wout_t")
            nc.sync.dma_start(wout_t[:],
                              io["wout"].rearrange("(hc p) d -> p hc d", p=128))
            w1r = bpool.tile([128, KC, KC, 128], f32r, name="w1r")
            w2r = bpool.tile([128, KC, KC, 128], f32r, name="w2r")
            for hcx in range(KC):
                nc.sync.dma_start(w1r[:, :, hcx, :],
                                  io["w1"][hcx].rearrange("(kc p) m -> p kc m", p=128))
                nc.sync.dma_start(w2r[:, :, hcx, :],
                                  io["w2"][hcx].rearrange("(kc p) m -> p kc m", p=128))
            for b in range(B):
                for sb in range(4):
                    hnT = hnt_pool.tile([128, KC, 512], f32r, tag="hnT", name="hnT")
                    for q4 in range(4):
                        st = b * 16 + sb * 4 + q4
                        def save_h(ht, g=slice(st * 128, (st + 1) * 128)):
                            nc.sync.dma_start(io["h_out"][g, :], ht[:])
                        norm_transpose(work, ar_out, hnT, st, q4 * 128,
                                       extra=save_h, src2=io["x"])
                    # router
                    plog = P((E, 512))
                    for kc in range(KC):
                        nc.tensor.matmul(plog[:], cst["rw"][:, kc, :], hnT[:, kc, :],
                                         start=(kc == 0), stop=(kc == KC - 1))
                    lsb = work.tile([E, 512], f32, tag="lsb", name="lsb")
                    nc.scalar.copy(lsb[:], plog[:])
                    for q4 in range(4):
                        st = b * 16 + sb * 4 + q4
                        g = slice(st * 128, (st + 1) * 128)
                        ptr = P((128, E))
                        nc.tensor.transpose(ptr[:], lsb[:, q4 * 128:(q4 + 1) * 128],
                                            cst["ident"][0:E, 0:E])
                        nt = work.tile([128, E], f32, tag="nt", name="nt")
                        nc.sync.dma_start(nt[:], io["noise"][g, :])
                        zt = work.tile([128, E], f32, tag="zt", name="zt")
                        nc.vector.tensor_tensor(zt[:], ptr[:], nt[:], op=OP.add)
                        ez = work.tile([128, E], f32, tag="ez", name="ez")
                        den = work.tile([128, 1], f32, tag="den", name="den")
                        nc.scalar.activation(ez[:], zt[:], AF.Exp, accum_out=den[:])
                        rd = work.tile([128, 1], f32, tag="den", name="rd")
                        nc.vector.reciprocal(rd[:], den[:])
                        pr = work.tile([128, E], f32, tag="pr", name="pr")
                        nc.vector.tensor_scalar_mul(pr[:], ez[:], rd[:])
                        pet = work.tile([128, E], f32, tag="pet", name="pet")
                        nc.vector.tensor_tensor(pet[:], pr[:], cst["sel"][:],
                                                op=OP.mult)
                        pe = work.tile([128, 1], f32, tag="pe", name="pe")
                        nc.vector.reduce_sum(pe[:], pet[:], axis=AX.X)
                        gtt = work.tile([128, E], f32, tag="gtt", name="gtt")
                        nc.vector.tensor_scalar(gtt[:], pr[:], pe[:], None,
                                                op0=OP.is_gt)
                        cnt = work.tile([128, 1], f32, tag="cnt", name="cnt")
                        nc.vector.reduce_sum(cnt[:], gtt[:], axis=AX.X)
                        ind = work.tile([128, 1], f32, tag="cnt", name="ind")
                        nc.vector.tensor_single_scalar(ind[:], cnt[:], 1.5,
                                                       op=OP.is_lt)
                        nc.vector.tensor_tensor(wgt_all[:, st:st + 1], pe[:], ind[:],
                                                op=OP.mult)
                    # expert FFN
                    ht = moe.tile([128, KC, 512], f32r, tag="ht", name="ht", bufs=1)
                    for hc in range(KC):
                        p1 = P()
                        for kc in range(KC):
                            nc.tensor.matmul(p1[:], w1r[:, kc, hc, :], hnT[:, kc, :],
                                             start=(kc == 0), stop=(kc == KC - 1))
                        p2 = P()
                        for kc in range(KC):
                            nc.tensor.matmul(p2[:], w2r[:, kc, hc, :], hnT[:, kc, :],
                                             start=(kc == 0), stop=(kc == KC - 1))
                        s1 = work.tile([128, 512], f32, tag="s1", name="s1")
                        nc.scalar.activation(s1[:], p1[:], AF.Silu,
                                             bias=cst["b1h"][:, hc:hc + 1], scale=1.0)
                        nc.vector.scalar_tensor_tensor(
                            ht[:, hc, :], p2[:], cst["b2h"][:, hc:hc + 1], s1[:],
                            op0=OP.add, op1=OP.mult)
                    for q4 in range(4):
                        st = b * 16 + sb * 4 + q4
                        g = slice(st * 128, (st + 1) * 128)
                        for db in range(2):
                            peo = P()
                            for hc in range(KC):
                                nc.tensor.matmul(
                                    peo[:], ht[:, hc, q4 * 128:(q4 + 1) * 128],
                                    wout_t[:, hc, db * 512:(db + 1) * 512],
                                    start=(hc == 0), stop=(hc == KC - 1))
                            ob = work.tile([128, 512], f32, tag="ob", name="ob")
                            nc.vector.tensor_scalar_mul(ob[:], peo[:],
                                                        wgt_all[:, st:st + 1])
                            nc.sync.dma_start(io["out"][g, db * 512:(db + 1) * 512],
                                                ob[:])

    nc.compile()
    return nc


# =====================================================================
# Host-side input prep / output combine
# =====================================================================
def prep_in_maps(inputs):
    x = np.asarray(inputs["x"], np.float32).reshape(T, D)
    scale1 = np.asarray(inputs["scale1"], np.float32)
    scale2 = np.asarray(inputs["scale2"], np.float32)
    wq = scale1[:, None] * np.asarray(inputs["wq"], np.float32)
    wk = scale1[:, None] * np.asarray(inputs["wk"], np.float32)
    wv = scale1[:, None] * np.asarray(inputs["wv"], np.float32)
    wo = np.asarray(inputs["wo"], np.float32)
    rw = scale2[:, None] * np.asarray(inputs["router_w"], np.float32)
    w1 = scale2[None, :, None] * np.asarray(inputs["w1"], np.float32)
    w2 = scale2[None, :, None] * np.asarray(inputs["w2"], np.float32)
    wout = np.asarray(inputs["wout"], np.float32)
    b1 = np.asarray(inputs["b1"], np.float32)
    b2 = np.asarray(inputs["b2"], np.float32)

    import jax
    noise = np.asarray(jax.random.gumbel(jax.random.key(42), (B, S, E),
                                         np.float32)) * 0.05
    noise = noise.reshape(T, E).astype(np.float32)

    half = DK // 2
    inv = 1.0 / (10000.0 ** (np.arange(half, dtype=np.float32) / half))
    ang = np.arange(S, dtype=np.float32)[:, None] * inv[None, :]  # [S, 32]
    cos_h = np.cos(ang).T  # [32, S]
    sin_h = np.sin(ang).T
    blk_cos = np.concatenate([cos_h, cos_h], 0)        # [64, S]
    blk_sin = np.concatenate([sin_h, sin_h], 0)
    cosb = np.concatenate([blk_cos, blk_cos], 0).astype(np.float32)  # [128, S]
    sinb = np.concatenate([blk_sin, blk_sin], 0).astype(np.float32)

    masks = np.zeros((128, 4, 512), np.float32)
    kr = np.arange(128)[:, None]
    qc = np.arange(512)[None, :]
    for t in range(4):
        masks[:, t, :] = np.where(kr + 128 * t <= qc, 0.0, MASKNEG)

    ident = np.eye(128, dtype=np.float32)
    id64 = np.zeros((128, 128), np.float32)
    id64[64:128, 0:64] = np.eye(64, dtype=np.float32)

    in_maps = []
    for c in range(NC):
        cols = slice(c * 128, (c + 1) * 128)
        wq_c = np.ascontiguousarray(wq[:, cols])
        wk_c = np.ascontiguousarray(wk[:, cols])
        wv_c = np.ascontiguousarray(wv[:, cols])
        def swap(w):
            ws = np.empty_like(w)
            for hh in range(2):
                r = hh * 64
                ws[:, r:r + 32] = -w[:, r + 32:r + 64]
                ws[:, r + 32:r + 64] = w[:, r:r + 32]
            return ws
        w1_c = np.stack([np.ascontiguousarray(w1[c][:, i * 128:(i + 1) * 128])
                         for i in range(KC)], 0)
        w2_c = np.stack([np.ascontiguousarray(w2[c][:, i * 128:(i + 1) * 128])
                         for i in range(KC)], 0)
        sel = np.zeros((128, E), np.float32)
        sel[:, c] = 1.0
        in_maps.append({
            "x": x, "wq": wq_c, "wk": wk_c, "wv": wv_c,
            "wqs": swap(wq_c), "wks": swap(wk_c),
            "wo": np.ascontiguousarray(wo[cols, :]),
            "cosb": cosb, "sinb": sinb, "masks": masks,
            "ident": ident, "id64": id64,
            "rw": rw, "noise": noise, "sel": sel,
            "w1": w1_c, "w2": w2_c,
            "wout": np.ascontiguousarray(wout[c]),
            "b1h": np.ascontiguousarray(b1[c].reshape(KC, 128).T),
            "b2h": np.ascontiguousarray(b2[c].reshape(KC, 128).T),
        })
    return in_maps


def combine(results):
    y = results[0]["h_out"].astype(np.float64)
    for c in range(NC):
        y = y + results[c]["out"].astype(np.float64)
    return y.astype(np.float32).reshape(B, S, D)


# ---------------------------------------------------------------------
# PJRT runner (axon): persistent jitted executable for the SPMD launch.
# ---------------------------------------------------------------------
import jax
from jax.sharding import Mesh, PartitionSpec
from jax.experimental.shard_map import shard_map
from concourse import bass2jax

import numpy as np
import jax
from jax.sharding import Mesh, PartitionSpec
from jax.experimental.shard_map import shard_map
import concourse.bass as bass
import concourse.mybir as mybir
from concourse import bass2jax


def make_runner(nc, n_cores):
    bass2jax.install_neuronx_cc_hook()
    partition_name = nc.partition_id_tensor.name if nc.partition_id_tensor else None
    in_names, out_names, out_avals, zero_outs = [], [], [], []
    for alloc in nc.m.functions[0].allocations:
        if not isinstance(alloc, mybir.MemoryLocationSet):
            continue
        name = alloc.memorylocations[0].name
        if alloc.kind == "ExternalInput":
            if name != partition_name:
                in_names.append(name)
        elif alloc.kind == "ExternalOutput":
            out_names.append(name)
            shape = tuple(alloc.tensor_shape)
            dtype = mybir.dt.np(alloc.dtype)
            out_avals.append(jax.core.ShapedArray(shape, dtype))
            zero_outs.append(np.zeros(shape, dtype))
    n_params = len(in_names)
    n_outs = len(out_avals)
    all_in_names = list(in_names) + list(out_names)
    if partition_name is not None:
        all_in_names.append(partition_name)

    def _body(*args):
        operands = list(args)
        if partition_name is not None:
            operands.append(bass2jax.partition_id_tensor())
        outs = bass2jax._bass_exec_p.bind(
            *operands,
            out_avals=tuple(out_avals),
            in_names=tuple(all_in_names),
            out_names=tuple(out_names),
            lowering_input_output_aliases=(),
            sim_require_finite=True,
            sim_require_nnan=True,
            nc=nc,
        )
        return tuple(outs)

    devices = jax.devices()[:n_cores]
    mesh = Mesh(np.asarray(devices), ("core",))
    in_specs = (PartitionSpec("core"),) * (n_params + n_outs)
    out_specs = (PartitionSpec("core"),) * n_outs
    donate = tuple(range(n_params, n_params + n_outs))
    sharded = jax.jit(
        shard_map(_body, mesh=mesh, in_specs=in_specs, out_specs=out_specs,
                  check_rep=False),
        donate_argnums=donate, keep_unused=True,
    )

    def run(in_maps):
        per_core = [[np.asarray(m[name]) for name in in_names] for m in in_maps]
        concat_in = [np.concatenate([per_core[c][i] for c in range(n_cores)], axis=0)
                     for i in range(n_params)]
        concat_zeros = [np.zeros((n_cores * z.shape[0], *z.shape[1:]), z.dtype)
                        for z in zero_outs]
        out_arrs = sharded(*concat_in, *concat_zeros)
        out_arrs = [np.asarray(o) for o in out_arrs]
        return [
            {name: out_arrs[i].reshape(n_cores, *out_avals[i].shape)[c]
             for i, name in enumerate(out_names)}
            for c in range(n_cores)
        ]

    return run


_CACHE = {}


def kernel(**inputs):
    if "nc" not in _CACHE:
        _CACHE["nc"] = build_program()
        _CACHE["run"] = make_runner(_CACHE["nc"], NC)
    in_maps = prep_in_maps(inputs)
    results = _CACHE["run"](in_maps)
    return combine(results)



# revision 22
# speedup vs baseline: 1.8543x; 1.8543x over previous
"""Self-contained Trainium2 Bass kernel for nn_DariushLayer_14087492731059.

kernel(**inputs) takes the FULL unsharded inputs of reference.setup_inputs()
and returns the full [B, S, D] float32 output, computed across 8 NeuronCores.

Phase A: attention tensor-parallel over heads (2 heads/core), bf16 compute.
Boundary: two bf16 AllToAll collectives exchange per-head attention outputs
so that each core ends up owning a 512-token shard (tokens c*512..(c+1)*512).
Phase B: out-projection + residual + rmsnorm + router + ALL 8 experts run
token-sharded on each core's shard (dense MoE = every expert sees every
token, so token sharding does the same FLOPs as expert sharding but needs no
broadcast of activations).  Expert weights stream from HBM in bf16.
Router runs in fp32r for bit-accurate top-2 selection.
Host side: concatenate the 8 output shards.
"""

import numpy as np
import concourse.bass as bass
import concourse.tile as tile
from concourse import bacc, mybir
from contextlib import ExitStack

f32, f32r, bf16 = mybir.dt.float32, mybir.dt.float32r, mybir.dt.bfloat16
AF = mybir.ActivationFunctionType
OP = mybir.AluOpType
AX = mybir.AxisListType

B, S, D, H, DK, E = 2, 2048, 1024, 16, 64, 8
T = B * S
NC = 8
KC = D // 128
SH = T // NC          # tokens per core shard (512)
HS = SH // 2          # half shard (256)
EPS = 1e-6


def build_program():
    nc = bacc.Bacc("TRN2", target_bir_lowering=False, debug=False, num_devices=NC)
    dt = nc.dram_tensor
    io = {}
    def inp(nm, shp, ty=f32):
        io[nm] = dt(nm, shp, ty, kind="ExternalInput").ap()
    def outp(nm, shp, ty=f32):
        io[nm] = dt(nm, shp, ty, kind="ExternalOutput").ap()

    inp("x", [T, D])                       # full input (phase A norm)
    inp("xsh", [SH, D])                    # this core's token shard of x
    inp("nsh", [SH, E])                    # gumbel noise shard
    for nm in ("wq", "wk", "wv", "wqs", "wks"):
        inp(nm, [128, KC, 128], bf16)      # head-sliced qkv weights [p,kc,m]
    inp("wo", [128, NC, D], bf16)          # full out-proj, src-core-major rows
    inp("cosb", [128, S], bf16); inp("sinb", [128, S], bf16)
    inp("masks01", [128, 4, 512], bf16)    # multiplicative causal masks
    inp("identb", [128, 128], bf16); inp("id64b", [128, 128], bf16)
    inp("ident32", [128, 128], f32r)
    inp("rwt", [128, KC, E], f32r)         # router weights [p,kc,e]
    inp("w1t", [E, 128, KC, D], bf16)      # per-expert gate proj [p,kc,hc*128+m]
    inp("w2t", [E, 128, KC, D], bf16)
    inp("woutt", [E, 128, KC, D], bf16)    # per-expert out proj [p,hc,m]
    inp("b1h", [128, E * KC]); inp("b2h", [128, E * KC])
    outp("out", [SH, D])

    with tile.TileContext(nc) as tc, ExitStack() as top:
        const = top.enter_context(tc.tile_pool(name="const", bufs=1))
        psum = top.enter_context(tc.tile_pool(name="psum", bufs=8, space="PSUM"))
        dram = top.enter_context(tc.tile_pool(name="dram", bufs=1, space="DRAM"))

        def P(shape=(128, 512), ty=f32):
            return psum.tile(list(shape), ty, tag="ps", name="ps")

        cst = {}
        for nm, shp, ty in [("identb", [128, 128], bf16),
                            ("id64b", [128, 128], bf16),
                            ("ident32", [128, 128], f32r),
                            ("rwt", [128, KC, E], f32r),
                            ("b1h", [128, E * KC], f32),
                            ("b2h", [128, E * KC], f32)]:
            cst[nm] = const.tile(shp, ty, name=nm)
            nc.sync.dma_start(cst[nm][:], io[nm][:])
        eps_t = const.tile([128, 1], f32, name="eps_t")
        nc.vector.memset(eps_t[:], EPS)

        a2a_in = [dram.tile([NC, 128, HS], bf16, name=f"a2a_in{i}")
                  for i in range(2)]
        a2a_out = [dram.tile([NC, 128, HS], bf16, name=f"a2a_out{i}")
                   for i in range(2)]

        # --- rmsnorm one [128, D] fp32 row-tile already in SBUF -> rr [128,1]
        # (sq_scratch is an overwritten-later [128, D] tile reused as the
        #  Square output buffer)
        def rms_rr(work, xt, sq_scratch):
            ssum = work.tile([128, 1], f32, tag="ssum", name="ssum")
            nc.scalar.activation(sq_scratch[:], xt[:], AF.Square,
                                 accum_out=ssum[:])
            sd = work.tile([128, 1], f32, tag="ssum", name="sd")
            nc.scalar.activation(sd[:], ssum[:], AF.Sqrt, bias=eps_t[:],
                                 scale=1.0 / D)
            rr = work.tile([128, 1], f32, tag="ssum", name="rr")
            nc.vector.reciprocal(rr[:], sd[:])
            return rr

        # =================================================================
        # Phase A: attention (this core's 2 heads, all T tokens)
        # =================================================================
        with tc.tile_pool(name="qkv", bufs=1) as qkv, \
             tc.tile_pool(name="apool", bufs=1) as apool:
            qT = qkv.tile([128, T], bf16, name="qT")
            kT = qkv.tile([128, T], bf16, name="kT")
            vT = qkv.tile([128, T], bf16, name="vT")
            oT = qkv.tile([128, T], bf16, name="oT")
            for nm, shp, ty in [("cosb", [128, S], bf16), ("sinb", [128, S], bf16),
                                ("masks01", [128, 4, 512], bf16)]:
                cst[nm] = apool.tile(shp, ty, name=nm)
                nc.sync.dma_start(cst[nm][:], io[nm][:])
            for nm in ("wq", "wk", "wv", "wqs", "wks"):
                cst[nm] = apool.tile([128, KC, 128], bf16, name=nm)
                nc.sync.dma_start(cst[nm][:], io[nm][:])

            with tc.tile_pool(name="xnt", bufs=2) as xnt_pool, \
                 tc.tile_pool(name="work", bufs=3) as work:
                for b in range(B):
                    for sb in range(4):
                        xnT = xnt_pool.tile([128, KC, 512], bf16, tag="xnT",
                                            name="xnT")
                        for q4 in range(4):
                            st = b * 16 + sb * 4 + q4
                            r0 = st * 128
                            xt = work.tile([128, D], f32, tag="xt", name="xt")
                            nc.sync.dma_start(xt[:], io["x"][r0:r0 + 128, :])
                            xh = work.tile([128, D], bf16, tag="xh", name="xh")
                            rr = rms_rr(work, xt, xh)
                            nc.vector.tensor_scalar_mul(xh[:], xt[:], rr[:])
                            for kc in range(KC):
                                pt = P((128, 128), bf16)
                                nc.tensor.transpose(
                                    pt[:], xh[:, kc * 128:(kc + 1) * 128],
                                    cst["identb"][:])
                                nc.vector.tensor_copy(
                                    xnT[:, kc, q4 * 128:(q4 + 1) * 128], pt[:])
                        gl = slice(b * S + sb * 512, b * S + (sb + 1) * 512)
                        sl = slice(sb * 512, (sb + 1) * 512)
                        for base, swp, dst in (("wq", "wqs", qT),
                                               ("wk", "wks", kT)):
                            pa = P()
                            for kc in range(KC):
                                nc.tensor.matmul(pa[:], cst[base][:, kc, :],
                                                 xnT[:, kc, :],
                                                 start=(kc == 0),
                                                 stop=(kc == KC - 1))
                            pb = P()
                            for kc in range(KC):
                                nc.tensor.matmul(pb[:], cst[swp][:, kc, :],
                                                 xnT[:, kc, :],
                                                 start=(kc == 0),
                                                 stop=(kc == KC - 1))
                            t1 = work.tile([128, 512], f32, tag="t1", name="t1")
                            nc.vector.tensor_tensor(t1[:], pa[:],
                                                    cst["cosb"][:, sl],
                                                    op=OP.mult)
                            t2 = work.tile([128, 512], f32, tag="t2", name="t2")
                            nc.vector.tensor_tensor(t2[:], pb[:],
                                                    cst["sinb"][:, sl],
                                                    op=OP.mult)
                            nc.gpsimd.tensor_tensor(dst[:, gl], t1[:], t2[:],
                                                    op=OP.add)
                        pv = P()
                        for kc in range(KC):
                            nc.tensor.matmul(pv[:], cst["wv"][:, kc, :],
                                             xnT[:, kc, :],
                                             start=(kc == 0), stop=(kc == KC - 1))
                        nc.scalar.copy(vT[:, gl], pv[:])

            # attention core
            with tc.tile_pool(name="att", bufs=3) as att, \
                 tc.tile_pool(name="expp", bufs=4) as expp, \
                 tc.tile_pool(name="vsb", bufs=18) as vsbp:
                for b in range(B):
                    for h in range(2):
                        hr = slice(h * 64, (h + 1) * 64)
                        idn = cst["identb"] if h == 0 else cst["id64b"]
                        vchunks = []
                        for m in range(16):
                            gk = slice(b * S + m * 128, b * S + (m + 1) * 128)
                            pt = P((128, 64), bf16)
                            nc.tensor.transpose(pt[:], vT[hr, gk], idn[hr, 0:64])
                            vs = vsbp.tile([128, 66], bf16, tag="vs", name="vs")
                            nc.vector.tensor_copy(vs[:, 0:64], pt[:])
                            nc.vector.memset(vs[:, 64:65], 1.0)
                            nc.vector.memset(vs[:, 65:66], 0.0)
                            vchunks.append(vs)
                        for jq in range(4):
                            gq = slice(b * S + jq * 512, b * S + (jq + 1) * 512)
                            nch = 4 * jq + 4
                            pos = [P((128, 66)) for _ in range(4)]
                            for m in range(nch):
                                gk = slice(b * S + m * 128,
                                           b * S + (m + 1) * 128)
                                pse = P()
                                nc.tensor.matmul(pse[:], kT[hr, gk], qT[hr, gq],
                                                 start=True, stop=True)
                                et = expp.tile([128, 512], bf16, tag="et",
                                               name="et")
                                nc.scalar.activation(et[:], pse[:], AF.Exp,
                                                     scale=0.125)
                                if m >= 4 * jq:
                                    nc.vector.tensor_tensor(
                                        et[:], et[:],
                                        cst["masks01"][:, m - 4 * jq, :],
                                        op=OP.mult)
                                for qt in range(4):
                                    nc.tensor.matmul(
                                        pos[qt][:],
                                        et[:, qt * 128:(qt + 1) * 128],
                                        vchunks[m][:],
                                        start=(m == 0), stop=(m == nch - 1))
                            for qt in range(4):
                                rcp = att.tile([128, 1], f32, tag="rcp",
                                               name="rcp")
                                nc.vector.reciprocal(rcp[:], pos[qt][:, 64:65])
                                opr = att.tile([128, 64], bf16, tag="opr",
                                               name="opr")
                                nc.vector.tensor_scalar_mul(
                                    opr[:], pos[qt][:, 0:64], rcp[:])
                                ptt = P((128, 128), bf16)
                                nc.tensor.transpose(ptt[0:64, :], opr[:],
                                                    cst["identb"][:])
                                g128 = slice(b * S + jq * 512 + qt * 128,
                                             b * S + jq * 512 + (qt + 1) * 128)
                                nc.vector.tensor_copy(oT[hr, g128],
                                                      ptt[0:64, :])

                # ship per-head attention outputs to token-shard owners
                for j in range(NC):
                    c0 = j * SH
                    nc.sync.dma_start(a2a_in[0][j], oT[:, c0:c0 + HS])
                    nc.sync.dma_start(a2a_in[1][j], oT[:, c0 + HS:c0 + SH])

        for i in range(2):
            nc.gpsimd.collective_compute(
                "AllToAll", OP.bypass, replica_groups=[list(range(NC))],
                ins=[a2a_in[i].opt()], outs=[a2a_out[i].opt()])

        # =================================================================
        # Phase B: out-proj + residual + norm + router + all experts,
        # on this core's 512-token shard.
        # =================================================================
        wpool = top.enter_context(tc.tile_pool(name="wts", bufs=2))
        bper = top.enter_context(tc.tile_pool(name="bper", bufs=1))

        wt = {}
        def load_expert(e):
            tiles = []
            for key in ("w1t", "w2t", "woutt"):
                t = wpool.tile([128, KC, D], bf16, tag=key, name=f"{key}{e}")
                nc.sync.dma_start(t[:], io[key][e])
                tiles.append(t)
            wt[e] = tiles

        # prefetch first two experts; these DMAs don't depend on the
        # collectives, so they stream during the AllToAll wait
        load_expert(0)
        load_expert(1)

        with tc.tile_pool(name="workb", bufs=3) as work:
            wo_t = bper.tile([128, NC, D], bf16, name="wo_t")
            nc.sync.dma_start(wo_t[:], io["wo"][:])
            acc = [bper.tile([128, D], f32, name=f"acc{q}") for q in range(4)]
            hnT = [bper.tile([128, KC, HS], bf16, name=f"hnT{i}")
                   for i in range(2)]
            wgt = bper.tile([128, 4 * E], f32, name="wgt")
            aot = [bper.tile([128, NC, HS], bf16, name=f"aot{i}")
                   for i in range(2)]
            for hs in range(2):
                nc.sync.dma_start(aot[hs][:],
                                  a2a_out[hs].rearrange("n p m -> p n m"))
                hnT32 = work.tile([128, KC, HS], f32r, tag="hnT32",
                                  name="hnT32", bufs=2)
                for q in range(2):
                    qg = hs * 2 + q
                    r0 = qg * 128
                    xst = work.tile([128, D], f32, tag="xst", name="xst",
                                    bufs=2)
                    nc.sync.dma_start(xst[:], io["xsh"][r0:r0 + 128, :])
                    for db in range(2):
                        pp = P()
                        for src in range(NC):
                            nc.tensor.matmul(
                                pp[:], aot[hs][:, src, q * 128:(q + 1) * 128],
                                wo_t[:, src, db * 512:(db + 1) * 512],
                                start=(src == 0), stop=(src == NC - 1))
                        nc.vector.tensor_tensor(
                            acc[qg][:, db * 512:(db + 1) * 512],
                            xst[:, db * 512:(db + 1) * 512], pp[:], op=OP.add)
                    xh32 = work.tile([128, D], f32r, tag="xh32", name="xh32",
                                     bufs=2)
                    rr = rms_rr(work, acc[qg], xh32)
                    nc.vector.tensor_scalar_mul(xh32[:], acc[qg][:], rr[:])
                    for kc in range(KC):
                        pt32 = P((128, 128), f32r)
                        nc.tensor.transpose(pt32[:],
                                            xh32[:, kc * 128:(kc + 1) * 128],
                                            cst["ident32"][:])
                        nc.vector.tensor_copy(
                            hnT32[:, kc, q * 128:(q + 1) * 128], pt32[:])
                    nc.scalar.copy(hnT[hs][:, :, q * 128:(q + 1) * 128],
                                   hnT32[:, :, q * 128:(q + 1) * 128])
                # router (fp32r for exact top-2)
                plog = P((E, HS))
                for kc in range(KC):
                    nc.tensor.matmul(plog[:], cst["rwt"][:, kc, :],
                                     hnT32[:, kc, :],
                                     start=(kc == 0), stop=(kc == KC - 1))
                lsb = work.tile([E, HS], f32r, tag="lsb", name="lsb")
                nc.scalar.copy(lsb[:], plog[:])
                for q in range(2):
                    qg = hs * 2 + q
                    ptr = P((128, E), f32r)
                    nc.tensor.transpose(ptr[:], lsb[:, q * 128:(q + 1) * 128],
                                        cst["ident32"][0:E, 0:E])
                    nt = work.tile([128, E], f32, tag="nt", name="nt")
                    nc.sync.dma_start(nt[:], io["nsh"][qg * 128:(qg + 1) * 128, :])
                    zt = work.tile([128, E], f32, tag="zt", name="zt")
                    nc.vector.tensor_tensor(zt[:], ptr[:], nt[:], op=OP.add)
                    ez = work.tile([128, E], f32, tag="ez", name="ez")
                    den = work.tile([128, 1], f32, tag="den", name="den")
                    nc.scalar.activation(ez[:], zt[:], AF.Exp,
                                         accum_out=den[:])
                    rd = work.tile([128, 1], f32, tag="den", name="rd")
                    nc.vector.reciprocal(rd[:], den[:])
                    pr = work.tile([128, E], f32, tag="pr", name="pr")
                    nc.vector.tensor_scalar_mul(pr[:], ez[:], rd[:])
                    m1 = work.tile([128, 1], f32, tag="m1", name="m1")
                    nc.vector.reduce_max(m1[:], pr[:], axis=AX.X)
                    eqm = work.tile([128, E], f32, tag="eqm", name="eqm")
                    nc.vector.tensor_scalar(eqm[:], pr[:], m1[:], None,
                                            op0=OP.is_ge)
                    msk = work.tile([128, E], f32, tag="msk", name="msk")
                    nc.vector.scalar_tensor_tensor(msk[:], eqm[:], -30000.0,
                                                   pr[:], op0=OP.mult,
                                                   op1=OP.add)
                    m2 = work.tile([128, 1], f32, tag="m1", name="m2")
                    nc.vector.reduce_max(m2[:], msk[:], axis=AX.X)
                    ind = work.tile([128, E], f32, tag="ind", name="ind")
                    nc.vector.tensor_scalar(ind[:], pr[:], m2[:], None,
                                            op0=OP.is_ge)
                    nc.vector.tensor_tensor(wgt[:, qg * E:(qg + 1) * E],
                                            pr[:], ind[:], op=OP.mult)

            # expert FFN sweep over the shard
            for e in range(E):
                if e >= 2:
                    load_expert(e)
                w1e, w2e, woe = wt[e]
                for hs in range(2):
                    peo = [P() for _ in range(4)]  # (q, db) output accumulators
                    for hc in range(KC):
                        p1 = P((128, HS))
                        for kc in range(KC):
                            nc.tensor.matmul(
                                p1[:], w1e[:, kc, hc * 128:(hc + 1) * 128],
                                hnT[hs][:, kc, :],
                                start=(kc == 0), stop=(kc == KC - 1))
                        p2 = P((128, HS))
                        for kc in range(KC):
                            nc.tensor.matmul(
                                p2[:], w2e[:, kc, hc * 128:(hc + 1) * 128],
                                hnT[hs][:, kc, :],
                                start=(kc == 0), stop=(kc == KC - 1))
                        s1 = work.tile([128, HS], bf16, tag="s1", name="s1")
                        nc.scalar.activation(
                            s1[:], p1[:], AF.Silu,
                            bias=cst["b1h"][:, e * KC + hc:e * KC + hc + 1],
                            scale=1.0)
                        htc = work.tile([128, HS], bf16, tag="htc", name="htc")
                        nc.vector.scalar_tensor_tensor(
                            htc[:], p2[:],
                            cst["b2h"][:, e * KC + hc:e * KC + hc + 1],
                            s1[:], op0=OP.add, op1=OP.mult)
                        for q in range(2):
                            for db in range(2):
                                nc.tensor.matmul(
                                    peo[q * 2 + db][:],
                                    htc[:, q * 128:(q + 1) * 128],
                                    woe[:, hc, db * 512:(db + 1) * 512],
                                    start=(hc == 0), stop=(hc == KC - 1))
                    for q in range(2):
                        qg = hs * 2 + q
                        for db in range(2):
                            nc.vector.scalar_tensor_tensor(
                                acc[qg][:, db * 512:(db + 1) * 512],
                                peo[q * 2 + db][:],
                                wgt[:, qg * E + e:qg * E + e + 1],
                                acc[qg][:, db * 512:(db + 1) * 512],
                                op0=OP.mult, op1=OP.add)
            for qg in range(4):
                nc.sync.dma_start(io["out"][qg * 128:(qg + 1) * 128, :],
                                  acc[qg][:])

    nc.compile()
    return nc


# =====================================================================
# Host-side input prep / output combine
# =====================================================================
def prep_in_maps(inputs):
    np_bf16 = mybir.dt.np(bf16)
    x = np.asarray(inputs["x"], np.float32).reshape(T, D)
    scale1 = np.asarray(inputs["scale1"], np.float32)
    scale2 = np.asarray(inputs["scale2"], np.float32)
    wq = scale1[:, None] * np.asarray(inputs["wq"], np.float32)
    wk = scale1[:, None] * np.asarray(inputs["wk"], np.float32)
    wv = scale1[:, None] * np.asarray(inputs["wv"], np.float32)
    wo = np.asarray(inputs["wo"], np.float32)
    rw = scale2[:, None] * np.asarray(inputs["router_w"], np.float32)
    w1 = scale2[None, :, None] * np.asarray(inputs["w1"], np.float32)
    w2 = scale2[None, :, None] * np.asarray(inputs["w2"], np.float32)
    wout = np.asarray(inputs["wout"], np.float32)
    b1 = np.asarray(inputs["b1"], np.float32)
    b2 = np.asarray(inputs["b2"], np.float32)

    import jax
    noise = np.asarray(jax.random.gumbel(jax.random.key(42), (B, S, E),
                                         np.float32)) * 0.05
    noise = noise.reshape(T, E).astype(np.float32)

    half = DK // 2
    inv = 1.0 / (10000.0 ** (np.arange(half, dtype=np.float32) / half))
    ang = np.arange(S, dtype=np.float32)[:, None] * inv[None, :]  # [S, 32]
    cos_h = np.cos(ang).T  # [32, S]
    sin_h = np.sin(ang).T
    blk_cos = np.concatenate([cos_h, cos_h], 0)        # [64, S]
    blk_sin = np.concatenate([sin_h, sin_h], 0)
    cosb = np.concatenate([blk_cos, blk_cos], 0).astype(np_bf16)  # [128, S]
    sinb = np.concatenate([blk_sin, blk_sin], 0).astype(np_bf16)

    masks01 = np.zeros((128, 4, 512), np.float32)
    kr = np.arange(128)[:, None]
    qc = np.arange(512)[None, :]
    for t in range(4):
        masks01[:, t, :] = np.where(kr + 128 * t <= qc, 1.0, 0.0)
    masks01 = masks01.astype(np_bf16)

    identb = np.eye(128, dtype=np.float32).astype(np_bf16)
    id64b = np.zeros((128, 128), np.float32)
    id64b[64:128, 0:64] = np.eye(64, dtype=np.float32)
    id64b = id64b.astype(np_bf16)
    ident32 = np.eye(128, dtype=np.float32)

    def chunk_rows(w):  # [D, M] -> [128, KC, M] with rows = kc*128 + p
        return np.ascontiguousarray(
            w.reshape(KC, 128, w.shape[1]).transpose(1, 0, 2))

    rwt = chunk_rows(rw)                                 # [128, KC, E] fp32
    wo_t = np.ascontiguousarray(
        wo.reshape(NC, 128, D).transpose(1, 0, 2)).astype(np_bf16)
    w1t = np.stack([chunk_rows(w1[e]).astype(np_bf16) for e in range(E)], 0)
    w2t = np.stack([chunk_rows(w2[e]).astype(np_bf16) for e in range(E)], 0)
    woutt = np.stack([chunk_rows(wout[e]).astype(np_bf16) for e in range(E)], 0)
    b1h = np.concatenate([b1[e].reshape(KC, 128).T for e in range(E)],
                         1).astype(np.float32)           # [128, E*KC]
    b2h = np.concatenate([b2[e].reshape(KC, 128).T for e in range(E)],
                         1).astype(np.float32)

    def swap(w):
        ws = np.empty_like(w)
        for hh in range(2):
            r = hh * 64
            ws[:, r:r + 32] = -w[:, r + 32:r + 64]
            ws[:, r + 32:r + 64] = w[:, r:r + 32]
        return ws

    in_maps = []
    for c in range(NC):
        cols = slice(c * 128, (c + 1) * 128)
        wq_c = np.ascontiguousarray(wq[:, cols])
        wk_c = np.ascontiguousarray(wk[:, cols])
        wv_c = np.ascontiguousarray(wv[:, cols])
        in_maps.append({
            "x": x,
            "xsh": np.ascontiguousarray(x[c * SH:(c + 1) * SH]),
            "nsh": np.ascontiguousarray(noise[c * SH:(c + 1) * SH]),
            "wq": chunk_rows(wq_c).astype(np_bf16),
            "wk": chunk_rows(wk_c).astype(np_bf16),
            "wv": chunk_rows(wv_c).astype(np_bf16),
            "wqs": chunk_rows(swap(wq_c)).astype(np_bf16),
            "wks": chunk_rows(swap(wk_c)).astype(np_bf16),
            "wo": wo_t,
            "cosb": cosb, "sinb": sinb, "masks01": masks01,
            "identb": identb, "id64b": id64b, "ident32": ident32,
            "rwt": rwt,
            "w1t": w1t, "w2t": w2t, "woutt": woutt,
            "b1h": b1h, "b2h": b2h,
        })
    return in_maps


def combine(results):
    y = np.concatenate([results[c]["out"] for c in range(NC)], axis=0)
    return np.ascontiguousarray(y.astype(np.float32).reshape(B, S, D))


# ---------------------------------------------------------------------
# PJRT runner (axon): persistent jitted executable for the SPMD launch.
# ---------------------------------------------------------------------
import jax
from jax.sharding import Mesh, PartitionSpec
from jax.experimental.shard_map import shard_map
from concourse import bass2jax


def make_runner(nc, n_cores):
    bass2jax.install_neuronx_cc_hook()
    partition_name = nc.partition_id_tensor.name if nc.partition_id_tensor else None
    in_names, out_names, out_avals, zero_outs = [], [], [], []
    for alloc in nc.m.functions[0].allocations:
        if not isinstance(alloc, mybir.MemoryLocationSet):
            continue
        name = alloc.memorylocations[0].name
        if alloc.kind == "ExternalInput":
            if name != partition_name:
                in_names.append(name)
        elif alloc.kind == "ExternalOutput":
            out_names.append(name)
            shape = tuple(alloc.tensor_shape)
            dtype = mybir.dt.np(alloc.dtype)
            out_avals.append(jax.core.ShapedArray(shape, dtype))
            zero_outs.append(np.zeros(shape, dtype))
    n_params = len(in_names)
    n_outs = len(out_avals)
    all_in_names = list(in_names) + list(out_names)
    if partition_name is not None:
        all_in_names.append(partition_name)

    def _body(*args):
        operands = list(args)
        if partition_name is not None:
            operands.append(bass2jax.partition_id_tensor())
        outs = bass2jax._bass_exec_p.bind(
            *operands,
            out_avals=tuple(out_avals),
            in_names=tuple(all_in_names),
            out_names=tuple(out_names),
            lowering_input_output_aliases=(),
            sim_require_finite=True,
            sim_require_nnan=True,
            nc=nc,
        )
        return tuple(outs)

    devices = jax.devices()[:n_cores]
    mesh = Mesh(np.asarray(devices), ("core",))
    in_specs = (PartitionSpec("core"),) * (n_params + n_outs)
    out_specs = (PartitionSpec("core"),) * n_outs
    donate = tuple(range(n_params, n_params + n_outs))
    sharded = jax.jit(
        shard_map(_body, mesh=mesh, in_specs=in_specs, out_specs=out_specs,
                  check_rep=False),
        donate_argnums=donate, keep_unused=True,
    )

    def run(in_maps):
        per_core = [[np.asarray(m[name]) for name in in_names] for m in in_maps]
        concat_in = [np.concatenate([per_core[c][i] for c in range(n_cores)], axis=0)
                     for i in range(n_params)]
        concat_zeros = [np.zeros((n_cores * z.shape[0], *z.shape[1:]), z.dtype)
                        for z in zero_outs]
        out_arrs = sharded(*concat_in, *concat_zeros)
        out_arrs = [np.asarray(o) for o in out_arrs]
        return [
            {name: out_arrs[i].reshape(n_cores, *out_avals[i].shape)[c]
             for i, name in enumerate(out_names)}
            for c in range(n_cores)
        ]

    return run


_CACHE = {}


def kernel(**inputs):
    if "nc" not in _CACHE:
        _CACHE["nc"] = build_program()
        _CACHE["run"] = make_runner(_CACHE["nc"], NC)
    in_maps = prep_in_maps(inputs)
    results = _CACHE["run"](in_maps)
    return combine(results)


# revision 25
# speedup vs baseline: 1.8979x; 1.0235x over previous
"""Self-contained Trainium2 Bass kernel for nn_DariushLayer_14087492731059.

kernel(**inputs) takes the FULL unsharded inputs of reference.setup_inputs()
and returns the full [B, S, D] float32 output, computed across 8 NeuronCores.

Phase A: attention tensor-parallel over heads (2 heads/core), bf16 compute.
Boundary: two bf16 AllToAll collectives exchange per-head attention outputs
so that each core ends up owning a 512-token shard (tokens c*512..(c+1)*512).
Phase B: out-projection + residual + rmsnorm + router + ALL 8 experts run
token-sharded on each core's shard (dense MoE = every expert sees every
token, so token sharding does the same FLOPs as expert sharding but needs no
broadcast of activations).  Expert weights stream from HBM in bf16.
Router runs in fp32r for bit-accurate top-2 selection.
Host side: concatenate the 8 output shards.
"""

import numpy as np
import concourse.bass as bass
import concourse.tile as tile
from concourse import bacc, mybir
from contextlib import ExitStack

f32, f32r, bf16 = mybir.dt.float32, mybir.dt.float32r, mybir.dt.bfloat16
AF = mybir.ActivationFunctionType
OP = mybir.AluOpType
AX = mybir.AxisListType

B, S, D, H, DK, E = 2, 2048, 1024, 16, 64, 8
T = B * S
NC = 8
KC = D // 128
SH = T // NC          # tokens per core shard (512)
HS = SH // 2          # half shard (256)
EPS = 1e-6


def build_program():
    nc = bacc.Bacc("TRN2", target_bir_lowering=False, debug=False, num_devices=NC)
    dt = nc.dram_tensor
    io = {}
    def inp(nm, shp, ty=f32):
        io[nm] = dt(nm, shp, ty, kind="ExternalInput").ap()
    def outp(nm, shp, ty=f32):
        io[nm] = dt(nm, shp, ty, kind="ExternalOutput").ap()

    inp("x", [T, D])                       # full input (phase A norm)
    inp("xsh", [SH, D])                    # this core's token shard of x
    inp("nsh", [SH, E])                    # gumbel noise shard
    for nm in ("wq", "wk", "wv", "wqs", "wks"):
        inp(nm, [128, KC, 128], bf16)      # head-sliced qkv weights [p,kc,m]
    inp("wo", [128, NC, D], bf16)          # full out-proj, src-core-major rows
    inp("cosb", [128, S], bf16); inp("sinb", [128, S], bf16)
    inp("masks01", [128, 4, 512], bf16)    # multiplicative causal masks
    inp("identb", [128, 128], bf16); inp("id64b", [128, 128], bf16)
    inp("ident32", [128, 128], f32r)
    inp("rwt", [128, KC, E], f32r)         # router weights [p,kc,e]
    inp("w1t", [E, 128, KC, D], bf16)      # per-expert gate proj [p,kc,hc*128+m]
    inp("w2t", [E, 128, KC, D], bf16)
    inp("woutt", [E, 128, KC, D], bf16)    # per-expert out proj [p,hc,m]
    inp("b1h", [128, E * KC]); inp("b2h", [128, E * KC])
    outp("out", [SH, D])

    with tile.TileContext(nc) as tc, ExitStack() as top:
        const = top.enter_context(tc.tile_pool(name="const", bufs=1))
        psum = top.enter_context(tc.tile_pool(name="psum", bufs=8, space="PSUM"))
        dram = top.enter_context(tc.tile_pool(name="dram", bufs=1, space="DRAM"))

        def P(shape=(128, 512), ty=f32):
            return psum.tile(list(shape), ty, tag="ps", name="ps")

        cst = {}
        for nm, shp, ty in [("identb", [128, 128], bf16),
                            ("id64b", [128, 128], bf16),
                            ("ident32", [128, 128], f32r),
                            ("rwt", [128, KC, E], f32r),
                            ("b1h", [128, E * KC], f32),
                            ("b2h", [128, E * KC], f32)]:
            cst[nm] = const.tile(shp, ty, name=nm)
            nc.sync.dma_start(cst[nm][:], io[nm][:])
        eps_t = const.tile([128, 1], f32, name="eps_t")
        nc.vector.memset(eps_t[:], EPS)

        a2a_in = [dram.tile([NC, 128, HS], bf16, name=f"a2a_in{i}")
                  for i in range(2)]
        a2a_out = [dram.tile([NC, 128, HS], bf16, name=f"a2a_out{i}")
                   for i in range(2)]

        # --- rmsnorm one [128, D] fp32 row-tile already in SBUF -> rr [128,1]
        # (sq_scratch is an overwritten-later [128, D] tile reused as the
        #  Square output buffer)
        def rms_rr(work, xt, sq_scratch):
            ssum = work.tile([128, 1], f32, tag="ssum", name="ssum")
            nc.scalar.activation(sq_scratch[:], xt[:], AF.Square,
                                 accum_out=ssum[:])
            sd = work.tile([128, 1], f32, tag="ssum", name="sd")
            nc.scalar.activation(sd[:], ssum[:], AF.Sqrt, bias=eps_t[:],
                                 scale=1.0 / D)
            rr = work.tile([128, 1], f32, tag="ssum", name="rr")
            nc.vector.reciprocal(rr[:], sd[:])
            return rr

        # =================================================================
        # Phase A: attention (this core's 2 heads, all T tokens)
        # =================================================================
        with tc.tile_pool(name="qkv", bufs=1) as qkv, \
             tc.tile_pool(name="apool", bufs=1) as apool:
            qT = qkv.tile([128, T], bf16, name="qT")
            kT = qkv.tile([128, T], bf16, name="kT")
            vT = qkv.tile([128, T], bf16, name="vT")
            oT = qkv.tile([128, T], bf16, name="oT")
            for nm, shp, ty in [("cosb", [128, S], bf16), ("sinb", [128, S], bf16),
                                ("masks01", [128, 4, 512], bf16)]:
                cst[nm] = apool.tile(shp, ty, name=nm)
                nc.sync.dma_start(cst[nm][:], io[nm][:])
            for nm in ("wq", "wk", "wv", "wqs", "wks"):
                cst[nm] = apool.tile([128, KC, 128], bf16, name=nm)
                nc.sync.dma_start(cst[nm][:], io[nm][:])

            with tc.tile_pool(name="xnt", bufs=2) as xnt_pool, \
                 tc.tile_pool(name="work", bufs=3) as work:
                for b in range(B):
                    for sb in range(4):
                        xnT = xnt_pool.tile([128, KC, 512], bf16, tag="xnT",
                                            name="xnT")
                        for q4 in range(4):
                            st = b * 16 + sb * 4 + q4
                            r0 = st * 128
                            xt = work.tile([128, D], f32, tag="xt", name="xt")
                            nc.sync.dma_start(xt[:], io["x"][r0:r0 + 128, :])
                            xh = work.tile([128, D], bf16, tag="xh", name="xh")
                            rr = rms_rr(work, xt, xh)
                            nc.vector.tensor_scalar_mul(xh[:], xt[:], rr[:])
                            for kg in range(2):
                                pt = P((128, 512), bf16)
                                for kk in range(4):
                                    kc = kg * 4 + kk
                                    nc.tensor.transpose(
                                        pt[:, kk * 128:(kk + 1) * 128],
                                        xh[:, kc * 128:(kc + 1) * 128],
                                        cst["identb"][:])
                                nc.vector.tensor_copy(
                                    xnT[:, kg * 4:(kg + 1) * 4,
                                        q4 * 128:(q4 + 1) * 128],
                                    pt[:].rearrange("p (k m) -> p k m", k=4))
                        gl = slice(b * S + sb * 512, b * S + (sb + 1) * 512)
                        sl = slice(sb * 512, (sb + 1) * 512)
                        for base, swp, dst in (("wq", "wqs", qT),
                                               ("wk", "wks", kT)):
                            pa = P()
                            for kc in range(KC):
                                nc.tensor.matmul(pa[:], cst[base][:, kc, :],
                                                 xnT[:, kc, :],
                                                 start=(kc == 0),
                                                 stop=(kc == KC - 1))
                            pb = P()
                            for kc in range(KC):
                                nc.tensor.matmul(pb[:], cst[swp][:, kc, :],
                                                 xnT[:, kc, :],
                                                 start=(kc == 0),
                                                 stop=(kc == KC - 1))
                            t1 = work.tile([128, 512], f32, tag="t1", name="t1")
                            nc.vector.tensor_tensor(t1[:], pa[:],
                                                    cst["cosb"][:, sl],
                                                    op=OP.mult)
                            t2 = work.tile([128, 512], f32, tag="t2", name="t2")
                            nc.vector.tensor_tensor(t2[:], pb[:],
                                                    cst["sinb"][:, sl],
                                                    op=OP.mult)
                            nc.gpsimd.tensor_tensor(dst[:, gl], t1[:], t2[:],
                                                    op=OP.add)
                        pv = P()
                        for kc in range(KC):
                            nc.tensor.matmul(pv[:], cst["wv"][:, kc, :],
                                             xnT[:, kc, :],
                                             start=(kc == 0), stop=(kc == KC - 1))
                        nc.scalar.copy(vT[:, gl], pv[:])

            # attention core
            with tc.tile_pool(name="att", bufs=3) as att, \
                 tc.tile_pool(name="expp", bufs=4) as expp, \
                 tc.tile_pool(name="vsb", bufs=18) as vsbp:
                for b in range(B):
                    for h in range(2):
                        hr = slice(h * 64, (h + 1) * 64)
                        idn = cst["identb"] if h == 0 else cst["id64b"]
                        vchunks = []
                        for m in range(16):
                            gk = slice(b * S + m * 128, b * S + (m + 1) * 128)
                            pt = P((128, 64), bf16)
                            nc.tensor.transpose(pt[:], vT[hr, gk], idn[hr, 0:64])
                            vs = vsbp.tile([128, 66], bf16, tag="vs", name="vs")
                            nc.vector.tensor_copy(vs[:, 0:64], pt[:])
                            nc.vector.memset(vs[:, 64:65], 1.0)
                            nc.vector.memset(vs[:, 65:66], 0.0)
                            vchunks.append(vs)
                        for jq in range(4):
                            gq = slice(b * S + jq * 512, b * S + (jq + 1) * 512)
                            nch = 4 * jq + 4
                            pos = [P((128, 66)) for _ in range(4)]
                            # 2-deep software pipeline: score matmul + exp for
                            # step m+2 issue before the AV matmuls of step m,
                            # so the PE never waits on the Act/DVE chain.
                            ets = {}
                            for m in range(nch + 2):
                                if m < nch:
                                    gk = slice(b * S + m * 128,
                                               b * S + (m + 1) * 128)
                                    pse = P()
                                    nc.tensor.matmul(pse[:], kT[hr, gk],
                                                     qT[hr, gq],
                                                     start=True, stop=True)
                                    et = expp.tile([128, 512], bf16, tag="et",
                                                   name="et")
                                    nc.scalar.activation(et[:], pse[:], AF.Exp,
                                                         scale=0.125)
                                    if m >= 4 * jq:
                                        nc.vector.tensor_tensor(
                                            et[:], et[:],
                                            cst["masks01"][:, m - 4 * jq, :],
                                            op=OP.mult)
                                    ets[m] = et
                                ma = m - 2
                                if ma >= 0:
                                    for qt in range(4):
                                        nc.tensor.matmul(
                                            pos[qt][:],
                                            ets[ma][:, qt * 128:(qt + 1) * 128],
                                            vchunks[ma][:],
                                            start=(ma == 0),
                                            stop=(ma == nch - 1))
                                    del ets[ma]
                            for qt in range(4):
                                rcp = att.tile([128, 1], f32, tag="rcp",
                                               name="rcp")
                                nc.vector.reciprocal(rcp[:], pos[qt][:, 64:65])
                                opr = att.tile([128, 64], bf16, tag="opr",
                                               name="opr")
                                nc.vector.tensor_scalar_mul(
                                    opr[:], pos[qt][:, 0:64], rcp[:])
                                ptt = P((128, 128), bf16)
                                nc.tensor.transpose(ptt[0:64, :], opr[:],
                                                    cst["identb"][:])
                                g128 = slice(b * S + jq * 512 + qt * 128,
                                             b * S + jq * 512 + (qt + 1) * 128)
                                nc.vector.tensor_copy(oT[hr, g128],
                                                      ptt[0:64, :])

                # ship per-head attention outputs to token-shard owners
                for j in range(NC):
                    c0 = j * SH
                    nc.sync.dma_start(a2a_in[0][j], oT[:, c0:c0 + HS])
                    nc.sync.dma_start(a2a_in[1][j], oT[:, c0 + HS:c0 + SH])

        for i in range(2):
            nc.gpsimd.collective_compute(
                "AllToAll", OP.bypass, replica_groups=[list(range(NC))],
                ins=[a2a_in[i].opt()], outs=[a2a_out[i].opt()])

        # =================================================================
        # Phase B: out-proj + residual + norm + router + all experts,
        # on this core's 512-token shard.
        # =================================================================
        wpool = top.enter_context(tc.tile_pool(name="wts", bufs=2))
        bper = top.enter_context(tc.tile_pool(name="bper", bufs=1))

        wt = {}
        def load_expert(e):
            tiles = []
            for key in ("w1t", "w2t", "woutt"):
                t = wpool.tile([128, KC, D], bf16, tag=key, name=f"{key}{e}")
                nc.sync.dma_start(t[:], io[key][e])
                tiles.append(t)
            wt[e] = tiles

        # prefetch first two experts; these DMAs don't depend on the
        # collectives, so they stream during the AllToAll wait
        load_expert(0)
        load_expert(1)

        with tc.tile_pool(name="workb", bufs=3) as work:
            wo_t = bper.tile([128, NC, D], bf16, name="wo_t")
            nc.sync.dma_start(wo_t[:], io["wo"][:])
            acc = [bper.tile([128, D], f32, name=f"acc{q}") for q in range(4)]
            hnT = [bper.tile([128, KC, HS], bf16, name=f"hnT{i}")
                   for i in range(2)]
            wgt = bper.tile([128, 4 * E], f32, name="wgt")
            aot = [bper.tile([128, NC, HS], bf16, name=f"aot{i}")
                   for i in range(2)]
            for hs in range(2):
                nc.sync.dma_start(aot[hs][:],
                                  a2a_out[hs].rearrange("n p m -> p n m"))
                hnT32 = work.tile([128, KC, HS], f32r, tag="hnT32",
                                  name="hnT32", bufs=2)
                for q in range(2):
                    qg = hs * 2 + q
                    r0 = qg * 128
                    xst = work.tile([128, D], f32, tag="xst", name="xst",
                                    bufs=2)
                    nc.sync.dma_start(xst[:], io["xsh"][r0:r0 + 128, :])
                    for db in range(2):
                        pp = P()
                        for src in range(NC):
                            nc.tensor.matmul(
                                pp[:], aot[hs][:, src, q * 128:(q + 1) * 128],
                                wo_t[:, src, db * 512:(db + 1) * 512],
                                start=(src == 0), stop=(src == NC - 1))
                        nc.vector.tensor_tensor(
                            acc[qg][:, db * 512:(db + 1) * 512],
                            xst[:, db * 512:(db + 1) * 512], pp[:], op=OP.add)
                    xh32 = work.tile([128, D], f32r, tag="xh32", name="xh32",
                                     bufs=2)
                    rr = rms_rr(work, acc[qg], xh32)
                    nc.vector.tensor_scalar_mul(xh32[:], acc[qg][:], rr[:])
                    for kg in range(2):
                        pt32 = P((128, 512), f32r)
                        for kk in range(4):
                            kc = kg * 4 + kk
                            nc.tensor.transpose(
                                pt32[:, kk * 128:(kk + 1) * 128],
                                xh32[:, kc * 128:(kc + 1) * 128],
                                cst["ident32"][:])
                        nc.vector.tensor_copy(
                            hnT32[:, kg * 4:(kg + 1) * 4,
                                  q * 128:(q + 1) * 128],
                            pt32[:].rearrange("p (k m) -> p k m", k=4))
                    nc.scalar.copy(hnT[hs][:, :, q * 128:(q + 1) * 128],
                                   hnT32[:, :, q * 128:(q + 1) * 128])
                # router (fp32r for exact top-2)
                plog = P((E, HS))
                for kc in range(KC):
                    nc.tensor.matmul(plog[:], cst["rwt"][:, kc, :],
                                     hnT32[:, kc, :],
                                     start=(kc == 0), stop=(kc == KC - 1))
                lsb = work.tile([E, HS], f32r, tag="lsb", name="lsb")
                nc.scalar.copy(lsb[:], plog[:])
                for q in range(2):
                    qg = hs * 2 + q
                    ptr = P((128, E), f32r)
                    nc.tensor.transpose(ptr[:], lsb[:, q * 128:(q + 1) * 128],
                                        cst["ident32"][0:E, 0:E])
                    nt = work.tile([128, E], f32, tag="nt", name="nt")
                    nc.sync.dma_start(nt[:], io["nsh"][qg * 128:(qg + 1) * 128, :])
                    zt = work.tile([128, E], f32, tag="zt", name="zt")
                    nc.vector.tensor_tensor(zt[:], ptr[:], nt[:], op=OP.add)
                    ez = work.tile([128, E], f32, tag="ez", name="ez")
                    den = work.tile([128, 1], f32, tag="den", name="den")
                    nc.scalar.activation(ez[:], zt[:], AF.Exp,
                                         accum_out=den[:])
                    rd = work.tile([128, 1], f32, tag="den", name="rd")
                    nc.vector.reciprocal(rd[:], den[:])
                    pr = work.tile([128, E], f32, tag="pr", name="pr")
                    nc.vector.tensor_scalar_mul(pr[:], ez[:], rd[:])
                    m1 = work.tile([128, 1], f32, tag="m1", name="m1")
                    nc.vector.reduce_max(m1[:], pr[:], axis=AX.X)
                    eqm = work.tile([128, E], f32, tag="eqm", name="eqm")
                    nc.vector.tensor_scalar(eqm[:], pr[:], m1[:], None,
                                            op0=OP.is_ge)
                    msk = work.tile([128, E], f32, tag="msk", name="msk")
                    nc.vector.scalar_tensor_tensor(msk[:], eqm[:], -30000.0,
                                                   pr[:], op0=OP.mult,
                                                   op1=OP.add)
                    m2 = work.tile([128, 1], f32, tag="m1", name="m2")
                    nc.vector.reduce_max(m2[:], msk[:], axis=AX.X)
                    ind = work.tile([128, E], f32, tag="ind", name="ind")
                    nc.vector.tensor_scalar(ind[:], pr[:], m2[:], None,
                                            op0=OP.is_ge)
                    nc.vector.tensor_tensor(wgt[:, qg * E:(qg + 1) * E],
                                            pr[:], ind[:], op=OP.mult)

            # expert FFN sweep over the shard
            for e in range(E):
                if e >= 2:
                    load_expert(e)
                w1e, w2e, woe = wt[e]
                for hs in range(2):
                    peo = [P() for _ in range(4)]  # (q, db) output accumulators
                    for hc in range(KC):
                        p1 = P((128, HS))
                        for kc in range(KC):
                            nc.tensor.matmul(
                                p1[:], w1e[:, kc, hc * 128:(hc + 1) * 128],
                                hnT[hs][:, kc, :],
                                start=(kc == 0), stop=(kc == KC - 1))
                        p2 = P((128, HS))
                        for kc in range(KC):
                            nc.tensor.matmul(
                                p2[:], w2e[:, kc, hc * 128:(hc + 1) * 128],
                                hnT[hs][:, kc, :],
                                start=(kc == 0), stop=(kc == KC - 1))
                        s1 = work.tile([128, HS], bf16, tag="s1", name="s1")
                        nc.scalar.activation(
                            s1[:], p1[:], AF.Silu,
                            bias=cst["b1h"][:, e * KC + hc:e * KC + hc + 1],
                            scale=1.0)
                        htc = work.tile([128, HS], bf16, tag="htc", name="htc")
                        nc.vector.scalar_tensor_tensor(
                            htc[:], p2[:],
                            cst["b2h"][:, e * KC + hc:e * KC + hc + 1],
                            s1[:], op0=OP.add, op1=OP.mult)
                        for q in range(2):
                            for db in range(2):
                                nc.tensor.matmul(
                                    peo[q * 2 + db][:],
                                    htc[:, q * 128:(q + 1) * 128],
                                    woe[:, hc, db * 512:(db + 1) * 512],
                                    start=(hc == 0), stop=(hc == KC - 1))
                    for q in range(2):
                        qg = hs * 2 + q
                        for db in range(2):
                            nc.vector.scalar_tensor_tensor(
                                acc[qg][:, db * 512:(db + 1) * 512],
                                peo[q * 2 + db][:],
                                wgt[:, qg * E + e:qg * E + e + 1],
                                acc[qg][:, db * 512:(db + 1) * 512],
                                op0=OP.mult, op1=OP.add)
            for qg in range(4):
                nc.sync.dma_start(io["out"][qg * 128:(qg + 1) * 128, :],
                                  acc[qg][:])

    nc.compile()
    return nc


# =====================================================================
# Host-side input prep / output combine
# =====================================================================
def prep_in_maps(inputs):
    np_bf16 = mybir.dt.np(bf16)
    x = np.asarray(inputs["x"], np.float32).reshape(T, D)
    scale1 = np.asarray(inputs["scale1"], np.float32)
    scale2 = np.asarray(inputs["scale2"], np.float32)
    wq = scale1[:, None] * np.asarray(inputs["wq"], np.float32)
    wk = scale1[:, None] * np.asarray(inputs["wk"], np.float32)
    wv = scale1[:, None] * np.asarray(inputs["wv"], np.float32)
    wo = np.asarray(inputs["wo"], np.float32)
    rw = scale2[:, None] * np.asarray(inputs["router_w"], np.float32)
    w1 = scale2[None, :, None] * np.asarray(inputs["w1"], np.float32)
    w2 = scale2[None, :, None] * np.asarray(inputs["w2"], np.float32)
    wout = np.asarray(inputs["wout"], np.float32)
    b1 = np.asarray(inputs["b1"], np.float32)
    b2 = np.asarray(inputs["b2"], np.float32)

    import jax
    noise = np.asarray(jax.random.gumbel(jax.random.key(42), (B, S, E),
                                         np.float32)) * 0.05
    noise = noise.reshape(T, E).astype(np.float32)

    half = DK // 2
    inv = 1.0 / (10000.0 ** (np.arange(half, dtype=np.float32) / half))
    ang = np.arange(S, dtype=np.float32)[:, None] * inv[None, :]  # [S, 32]
    cos_h = np.cos(ang).T  # [32, S]
    sin_h = np.sin(ang).T
    blk_cos = np.concatenate([cos_h, cos_h], 0)        # [64, S]
    blk_sin = np.concatenate([sin_h, sin_h], 0)
    cosb = np.concatenate([blk_cos, blk_cos], 0).astype(np_bf16)  # [128, S]
    sinb = np.concatenate([blk_sin, blk_sin], 0).astype(np_bf16)

    masks01 = np.zeros((128, 4, 512), np.float32)
    kr = np.arange(128)[:, None]
    qc = np.arange(512)[None, :]
    for t in range(4):
        masks01[:, t, :] = np.where(kr + 128 * t <= qc, 1.0, 0.0)
    masks01 = masks01.astype(np_bf16)

    identb = np.eye(128, dtype=np.float32).astype(np_bf16)
    id64b = np.zeros((128, 128), np.float32)
    id64b[64:128, 0:64] = np.eye(64, dtype=np.float32)
    id64b = id64b.astype(np_bf16)
    ident32 = np.eye(128, dtype=np.float32)

    def chunk_rows(w):  # [D, M] -> [128, KC, M] with rows = kc*128 + p
        return np.ascontiguousarray(
            w.reshape(KC, 128, w.shape[1]).transpose(1, 0, 2))

    rwt = chunk_rows(rw)                                 # [128, KC, E] fp32
    wo_t = np.ascontiguousarray(
        wo.reshape(NC, 128, D).transpose(1, 0, 2)).astype(np_bf16)
    w1t = np.stack([chunk_rows(w1[e]).astype(np_bf16) for e in range(E)], 0)
    w2t = np.stack([chunk_rows(w2[e]).astype(np_bf16) for e in range(E)], 0)
    woutt = np.stack([chunk_rows(wout[e]).astype(np_bf16) for e in range(E)], 0)
    b1h = np.concatenate([b1[e].reshape(KC, 128).T for e in range(E)],
                         1).astype(np.float32)           # [128, E*KC]
    b2h = np.concatenate([b2[e].reshape(KC, 128).T for e in range(E)],
                         1).astype(np.float32)

    def swap(w):
        ws = np.empty_like(w)
        for hh in range(2):
            r = hh * 64
            ws[:, r:r + 32] = -w[:, r + 32:r + 64]
            ws[:, r + 32:r + 64] = w[:, r:r + 32]
        return ws

    in_maps = []
    for c in range(NC):
        cols = slice(c * 128, (c + 1) * 128)
        wq_c = np.ascontiguousarray(wq[:, cols])
        wk_c = np.ascontiguousarray(wk[:, cols])
        wv_c = np.ascontiguousarray(wv[:, cols])
        in_maps.append({
            "x": x,
            "xsh": np.ascontiguousarray(x[c * SH:(c + 1) * SH]),
            "nsh": np.ascontiguousarray(noise[c * SH:(c + 1) * SH]),
            "wq": chunk_rows(wq_c).astype(np_bf16),
            "wk": chunk_rows(wk_c).astype(np_bf16),
            "wv": chunk_rows(wv_c).astype(np_bf16),
            "wqs": chunk_rows(swap(wq_c)).astype(np_bf16),
            "wks": chunk_rows(swap(wk_c)).astype(np_bf16),
            "wo": wo_t,
            "cosb": cosb, "sinb": sinb, "masks01": masks01,
            "identb": identb, "id64b": id64b, "ident32": ident32,
            "rwt": rwt,
            "w1t": w1t, "w2t": w2t, "woutt": woutt,
            "b1h": b1h, "b2h": b2h,
        })
    return in_maps


def combine(results):
    y = np.concatenate([results[c]["out"] for c in range(NC)], axis=0)
    return np.ascontiguousarray(y.astype(np.float32).reshape(B, S, D))


# ---------------------------------------------------------------------
# PJRT runner (axon): persistent jitted executable for the SPMD launch.
# ---------------------------------------------------------------------
import jax
from jax.sharding import Mesh, PartitionSpec
from jax.experimental.shard_map import shard_map
from concourse import bass2jax


def make_runner(nc, n_cores):
    bass2jax.install_neuronx_cc_hook()
    partition_name = nc.partition_id_tensor.name if nc.partition_id_tensor else None
    in_names, out_names, out_avals, zero_outs = [], [], [], []
    for alloc in nc.m.functions[0].allocations:
        if not isinstance(alloc, mybir.MemoryLocationSet):
            continue
        name = alloc.memorylocations[0].name
        if alloc.kind == "ExternalInput":
            if name != partition_name:
                in_names.append(name)
        elif alloc.kind == "ExternalOutput":
            out_names.append(name)
            shape = tuple(alloc.tensor_shape)
            dtype = mybir.dt.np(alloc.dtype)
            out_avals.append(jax.core.ShapedArray(shape, dtype))
            zero_outs.append(np.zeros(shape, dtype))
    n_params = len(in_names)
    n_outs = len(out_avals)
    all_in_names = list(in_names) + list(out_names)
    if partition_name is not None:
        all_in_names.append(partition_name)

    def _body(*args):
        operands = list(args)
        if partition_name is not None:
            operands.append(bass2jax.partition_id_tensor())
        outs = bass2jax._bass_exec_p.bind(
            *operands,
            out_avals=tuple(out_avals),
            in_names=tuple(all_in_names),
            out_names=tuple(out_names),
            lowering_input_output_aliases=(),
            sim_require_finite=True,
            sim_require_nnan=True,
            nc=nc,
        )
        return tuple(outs)

    devices = jax.devices()[:n_cores]
    mesh = Mesh(np.asarray(devices), ("core",))
    in_specs = (PartitionSpec("core"),) * (n_params + n_outs)
    out_specs = (PartitionSpec("core"),) * n_outs
    donate = tuple(range(n_params, n_params + n_outs))
    sharded = jax.jit(
        shard_map(_body, mesh=mesh, in_specs=in_specs, out_specs=out_specs,
                  check_rep=False),
        donate_argnums=donate, keep_unused=True,
    )

    def run(in_maps):
        per_core = [[np.asarray(m[name]) for name in in_names] for m in in_maps]
        concat_in = [np.concatenate([per_core[c][i] for c in range(n_cores)], axis=0)
                     for i in range(n_params)]
        concat_zeros = [np.zeros((n_cores * z.shape[0], *z.shape[1:]), z.dtype)
                        for z in zero_outs]
        out_arrs = sharded(*concat_in, *concat_zeros)
        out_arrs = [np.asarray(o) for o in out_arrs]
        return [
            {name: out_arrs[i].reshape(n_cores, *out_avals[i].shape)[c]
             for i, name in enumerate(out_names)}
            for c in range(n_cores)
        ]

    return run


_CACHE = {}


def kernel(**inputs):
    if "nc" not in _CACHE:
        _CACHE["nc"] = build_program()
        _CACHE["run"] = make_runner(_CACHE["nc"], NC)
    in_maps = prep_in_maps(inputs)
    results = _CACHE["run"](in_maps)
    return combine(results)


# revision 31
# speedup vs baseline: 2.0902x; 1.1013x over previous
"""Self-contained Trainium2 Bass kernel for nn_DariushLayer_14087492731059.

kernel(**inputs) takes the FULL unsharded inputs of reference.setup_inputs()
and returns the full [B, S, D] float32 output, computed across 8 NeuronCores.

Phase A: attention tensor-parallel over heads (2 heads/core), bf16 compute.
Boundary: two bf16 AllToAll collectives exchange per-head attention outputs
so that each core ends up owning a 512-token shard (tokens c*512..(c+1)*512).
Phase B: out-projection + residual + rmsnorm + router + ALL 8 experts run
token-sharded on each core's shard (dense MoE = every expert sees every
token, so token sharding does the same FLOPs as expert sharding but needs no
broadcast of activations).  Expert weights stream from HBM in bf16.
Router runs in fp32r for bit-accurate top-2 selection.
Host side: concatenate the 8 output shards.
"""

import numpy as np
import concourse.bass as bass
import concourse.tile as tile
from concourse import bacc, mybir
from contextlib import ExitStack

f32, f32r, bf16 = mybir.dt.float32, mybir.dt.float32r, mybir.dt.bfloat16
AF = mybir.ActivationFunctionType
OP = mybir.AluOpType
AX = mybir.AxisListType

B, S, D, H, DK, E = 2, 2048, 1024, 16, 64, 8
T = B * S
NC = 8
KC = D // 128
SH = T // NC          # tokens per core shard (512)
HS = SH // 2          # half shard (256)
EPS = 1e-6


def build_program():
    nc = bacc.Bacc("TRN2", target_bir_lowering=False, debug=False, num_devices=NC)
    dt = nc.dram_tensor
    io = {}
    def inp(nm, shp, ty=f32):
        io[nm] = dt(nm, shp, ty, kind="ExternalInput").ap()
    def outp(nm, shp, ty=f32):
        io[nm] = dt(nm, shp, ty, kind="ExternalOutput").ap()

    inp("x", [T, D])                       # full input (phase A norm)
    inp("xsh", [SH, D])                    # this core's token shard of x
    inp("nsh", [SH, E])                    # gumbel noise shard
    for nm in ("wq", "wk", "wv", "wqs", "wks"):
        inp(nm, [128, KC, 128], bf16)      # head-sliced qkv weights [p,kc,m]
    inp("wo", [128, NC, D], bf16)          # full out-proj, src-core-major rows
    inp("cosb", [128, S], bf16); inp("sinb", [128, S], bf16)
    inp("masks01", [128, 4, 512], bf16)    # multiplicative causal masks
    inp("identb", [128, 128], bf16); inp("id64b", [128, 128], bf16)
    inp("ident32", [128, 128], f32r)
    inp("rwt", [128, KC, E], f32r)         # router weights [p,kc,e]
    inp("w1t", [E, 128, KC, D], bf16)      # per-expert gate proj [p,kc,hc*128+m]
    inp("w2t", [E, 128, KC, D], bf16)
    inp("woutt", [E, 128, KC, D], bf16)    # per-expert out proj [p,hc,m]
    inp("b1h", [128, E * KC]); inp("b2h", [128, E * KC])
    outp("out", [SH, D])

    with tile.TileContext(nc) as tc, ExitStack() as top:
        const = top.enter_context(tc.tile_pool(name="const", bufs=1))
        psum = top.enter_context(tc.tile_pool(name="psum", bufs=8, space="PSUM"))
        dram = top.enter_context(tc.tile_pool(name="dram", bufs=1, space="DRAM"))

        def P(shape=(128, 512), ty=f32):
            return psum.tile(list(shape), ty, tag="ps", name="ps")

        cst = {}
        for nm, shp, ty in [("identb", [128, 128], bf16),
                            ("id64b", [128, 128], bf16),
                            ("ident32", [128, 128], f32r),
                            ("rwt", [128, KC, E], f32r),
                            ("b1h", [128, E * KC], f32),
                            ("b2h", [128, E * KC], f32)]:
            cst[nm] = const.tile(shp, ty, name=nm)
            nc.sync.dma_start(cst[nm][:], io[nm][:])
        eps_t = const.tile([128, 1], f32, name="eps_t")
        nc.vector.memset(eps_t[:], EPS)

        a2a_in = [dram.tile([NC, 128, HS], bf16, name=f"a2a_in{i}")
                  for i in range(2)]
        a2a_out = [dram.tile([NC, 128, HS], bf16, name=f"a2a_out{i}")
                   for i in range(2)]

        # --- rmsnorm one [128, D] fp32 row-tile already in SBUF -> rr [128,1]
        # (sq_scratch is an overwritten-later [128, D] tile reused as the
        #  Square output buffer)
        def rms_rr(work, xt, sq_scratch):
            ssum = work.tile([128, 1], f32, tag="ssum", name="ssum")
            nc.scalar.activation(sq_scratch[:], xt[:], AF.Square,
                                 accum_out=ssum[:])
            sd = work.tile([128, 1], f32, tag="ssum", name="sd")
            nc.scalar.activation(sd[:], ssum[:], AF.Sqrt, bias=eps_t[:],
                                 scale=1.0 / D)
            rr = work.tile([128, 1], f32, tag="ssum", name="rr")
            nc.vector.reciprocal(rr[:], sd[:])
            return rr

        # =================================================================
        # Phase A: attention (this core's 2 heads, all T tokens)
        # =================================================================
        with tc.tile_pool(name="qkv", bufs=1) as qkv, \
             tc.tile_pool(name="apool", bufs=1) as apool:
            qT = qkv.tile([128, T], bf16, name="qT")
            kT = qkv.tile([128, T], bf16, name="kT")
            vT = qkv.tile([128, T], bf16, name="vT")
            oT = qkv.tile([128, T], bf16, name="oT")
            for nm, shp, ty in [("cosb", [128, S], bf16), ("sinb", [128, S], bf16),
                                ("masks01", [128, 4, 512], bf16)]:
                cst[nm] = apool.tile(shp, ty, name=nm)
                nc.sync.dma_start(cst[nm][:], io[nm][:])
            for nm in ("wq", "wk", "wv", "wqs", "wks"):
                cst[nm] = apool.tile([128, KC, 128], bf16, name=nm)
                nc.sync.dma_start(cst[nm][:], io[nm][:])

            with tc.tile_pool(name="xnt", bufs=3) as xnt_pool, \
                 tc.tile_pool(name="work", bufs=4) as work:
                for b in range(B):
                    for sb in range(4):
                        xnT = xnt_pool.tile([128, KC, 512], bf16, tag="xnT",
                                            name="xnT")
                        for q4 in range(4):
                            st = b * 16 + sb * 4 + q4
                            r0 = st * 128
                            xt = work.tile([128, D], f32, tag="xt", name="xt")
                            nc.sync.dma_start(xt[:], io["x"][r0:r0 + 128, :])
                            xh = work.tile([128, D], bf16, tag="xh", name="xh")
                            rr = rms_rr(work, xt, xh)
                            nc.scalar.mul(xh[:], xt[:], rr[:])
                            for kg in range(2):
                                pt = P((128, 512), bf16)
                                for kk in range(4):
                                    kc = kg * 4 + kk
                                    nc.tensor.transpose(
                                        pt[:, kk * 128:(kk + 1) * 128],
                                        xh[:, kc * 128:(kc + 1) * 128],
                                        cst["identb"][:])
                                nc.vector.tensor_copy(
                                    xnT[:, kg * 4:(kg + 1) * 4,
                                        q4 * 128:(q4 + 1) * 128],
                                    pt[:].rearrange("p (k m) -> p k m", k=4))
                        gl = slice(b * S + sb * 512, b * S + (sb + 1) * 512)
                        sl = slice(sb * 512, (sb + 1) * 512)
                        for base, swp, dst in (("wq", "wqs", qT),
                                               ("wk", "wks", kT)):
                            pa = P()
                            for kc in range(KC):
                                nc.tensor.matmul(pa[:], cst[base][:, kc, :],
                                                 xnT[:, kc, :],
                                                 start=(kc == 0),
                                                 stop=(kc == KC - 1))
                            pb = P()
                            for kc in range(KC):
                                nc.tensor.matmul(pb[:], cst[swp][:, kc, :],
                                                 xnT[:, kc, :],
                                                 start=(kc == 0),
                                                 stop=(kc == KC - 1))
                            t1 = work.tile([128, 512], f32, tag="t1", name="t1")
                            nc.vector.tensor_tensor(t1[:], pa[:],
                                                    cst["cosb"][:, sl],
                                                    op=OP.mult)
                            t2 = work.tile([128, 512], f32, tag="t2", name="t2")
                            nc.vector.tensor_tensor(t2[:], pb[:],
                                                    cst["sinb"][:, sl],
                                                    op=OP.mult)
                            nc.gpsimd.tensor_tensor(dst[:, gl], t1[:], t2[:],
                                                    op=OP.add)
                        pv = P()
                        for kc in range(KC):
                            nc.tensor.matmul(pv[:], cst["wv"][:, kc, :],
                                             xnT[:, kc, :],
                                             start=(kc == 0), stop=(kc == KC - 1))
                        nc.scalar.copy(vT[:, gl], pv[:])

            # attention core
            with tc.tile_pool(name="att", bufs=3) as att, \
                 tc.tile_pool(name="expp", bufs=5) as expp, \
                 tc.tile_pool(name="vsb", bufs=18) as vsbp:
                for b in range(B):
                    for h in range(2):
                        hr = slice(h * 64, (h + 1) * 64)
                        idn = cst["identb"] if h == 0 else cst["id64b"]
                        vchunks = []
                        for m in range(16):
                            gk = slice(b * S + m * 128, b * S + (m + 1) * 128)
                            pt = P((128, 64), bf16)
                            nc.tensor.transpose(pt[:], vT[hr, gk], idn[hr, 0:64])
                            vs = vsbp.tile([128, 66], bf16, tag="vs", name="vs")
                            nc.vector.tensor_copy(vs[:, 0:64], pt[:])
                            nc.vector.memset(vs[:, 64:65], 1.0)
                            nc.vector.memset(vs[:, 65:66], 0.0)
                            vchunks.append(vs)
                        for jq in range(4):
                            gq = slice(b * S + jq * 512, b * S + (jq + 1) * 512)
                            nch = 4 * jq + 4
                            pos = [P((128, 66)) for _ in range(4)]
                            # 3-deep software pipeline: score matmul + exp for
                            # step m+3 issue before the AV matmuls of step m,
                            # so the PE never waits on the Act/DVE chain.
                            # Causal structure per 128x128 sub-block: for the
                            # diagonal key chunk (t = m-4jq) only the qt == t
                            # sub-block needs masking; qt < t sub-blocks are
                            # fully masked and their AV matmuls are skipped.
                            DEPTH = 3
                            ets = {}
                            for m in range(nch + DEPTH):
                                if m < nch:
                                    t = m - 4 * jq  # >=0 on the diagonal chunk
                                    lo = max(t, 0) * 128
                                    gk = slice(b * S + m * 128,
                                               b * S + (m + 1) * 128)
                                    pse = P()
                                    nc.tensor.matmul(pse[:], kT[hr, gk],
                                                     qT[hr, gq],
                                                     start=True, stop=True)
                                    et = expp.tile([128, 512], bf16, tag="et",
                                                   name="et")
                                    nc.scalar.activation(et[:, lo:512],
                                                         pse[:, lo:512],
                                                         AF.Exp, scale=0.125)
                                    if t >= 0:
                                        nc.vector.tensor_tensor(
                                            et[:, lo:lo + 128],
                                            et[:, lo:lo + 128],
                                            cst["masks01"][:, 0, 0:128],
                                            op=OP.mult)
                                    ets[m] = et
                                ma = m - DEPTH
                                if ma >= 0:
                                    ta = ma - 4 * jq
                                    for qt in range(max(ta, 0), 4):
                                        nc.tensor.matmul(
                                            pos[qt][:],
                                            ets[ma][:, qt * 128:(qt + 1) * 128],
                                            vchunks[ma][:],
                                            start=(ma == 0),
                                            stop=(ma == 4 * jq + qt))
                                    del ets[ma]
                            for qt in range(4):
                                rcp = att.tile([128, 1], f32, tag="rcp",
                                               name="rcp")
                                nc.vector.reciprocal(rcp[:], pos[qt][:, 64:65])
                                opr = att.tile([128, 64], bf16, tag="opr",
                                               name="opr")
                                nc.vector.tensor_scalar_mul(
                                    opr[:], pos[qt][:, 0:64], rcp[:])
                                ptt = P((128, 128), bf16)
                                nc.tensor.transpose(ptt[0:64, :], opr[:],
                                                    cst["identb"][:])
                                g128 = slice(b * S + jq * 512 + qt * 128,
                                             b * S + jq * 512 + (qt + 1) * 128)
                                nc.vector.tensor_copy(oT[hr, g128],
                                                      ptt[0:64, :])

                # ship per-head attention outputs to token-shard owners
                for j in range(NC):
                    c0 = j * SH
                    nc.sync.dma_start(a2a_in[0][j], oT[:, c0:c0 + HS])
                    nc.sync.dma_start(a2a_in[1][j], oT[:, c0 + HS:c0 + SH])

        for i in range(2):
            nc.gpsimd.collective_compute(
                "AllToAll", OP.bypass, replica_groups=[list(range(NC))],
                ins=[a2a_in[i].opt()], outs=[a2a_out[i].opt()])

        # =================================================================
        # Phase B: out-proj + residual + norm + router + all experts,
        # on this core's 512-token shard.
        # =================================================================
        wpool = top.enter_context(tc.tile_pool(name="wts", bufs=2))
        bper = top.enter_context(tc.tile_pool(name="bper", bufs=1))

        wt = {}
        def load_expert(e):
            tiles = []
            for key in ("w1t", "w2t", "woutt"):
                t = wpool.tile([128, KC, D], bf16, tag=key, name=f"{key}{e}")
                nc.sync.dma_start(t[:], io[key][e])
                tiles.append(t)
            wt[e] = tiles

        # prefetch first two experts; these DMAs don't depend on the
        # collectives, so they stream during the AllToAll wait
        load_expert(0)
        load_expert(1)

        with tc.tile_pool(name="workb", bufs=3) as work:
            wo_t = bper.tile([128, NC, D], bf16, name="wo_t")
            nc.sync.dma_start(wo_t[:], io["wo"][:])
            acc = [bper.tile([128, D], f32, name=f"acc{q}") for q in range(4)]
            hnT = [bper.tile([128, KC, HS], bf16, name=f"hnT{i}")
                   for i in range(2)]
            wgt = bper.tile([128, 4 * E], f32, name="wgt")
            aot = [bper.tile([128, NC, HS], bf16, name=f"aot{i}")
                   for i in range(2)]
            for hs in range(2):
                nc.sync.dma_start(aot[hs][:],
                                  a2a_out[hs].rearrange("n p m -> p n m"))
                hnT32 = work.tile([128, KC, HS], f32r, tag="hnT32",
                                  name="hnT32", bufs=2)
                for q in range(2):
                    qg = hs * 2 + q
                    r0 = qg * 128
                    xst = work.tile([128, D], f32, tag="xst", name="xst",
                                    bufs=2)
                    nc.sync.dma_start(xst[:], io["xsh"][r0:r0 + 128, :])
                    for db in range(2):
                        pp = P()
                        for src in range(NC):
                            nc.tensor.matmul(
                                pp[:], aot[hs][:, src, q * 128:(q + 1) * 128],
                                wo_t[:, src, db * 512:(db + 1) * 512],
                                start=(src == 0), stop=(src == NC - 1))
                        nc.vector.tensor_tensor(
                            acc[qg][:, db * 512:(db + 1) * 512],
                            xst[:, db * 512:(db + 1) * 512], pp[:], op=OP.add)
                    xh32 = work.tile([128, D], f32r, tag="xh32", name="xh32",
                                     bufs=2)
                    rr = rms_rr(work, acc[qg], xh32)
                    nc.vector.tensor_scalar_mul(xh32[:], acc[qg][:], rr[:])
                    for kg in range(2):
                        pt32 = P((128, 512), f32r)
                        for kk in range(4):
                            kc = kg * 4 + kk
                            nc.tensor.transpose(
                                pt32[:, kk * 128:(kk + 1) * 128],
                                xh32[:, kc * 128:(kc + 1) * 128],
                                cst["ident32"][:])
                        nc.vector.tensor_copy(
                            hnT32[:, kg * 4:(kg + 1) * 4,
                                  q * 128:(q + 1) * 128],
                            pt32[:].rearrange("p (k m) -> p k m", k=4))
                    nc.scalar.copy(hnT[hs][:, :, q * 128:(q + 1) * 128],
                                   hnT32[:, :, q * 128:(q + 1) * 128])
                # router (fp32r for exact top-2)
                plog = P((E, HS))
                for kc in range(KC):
                    nc.tensor.matmul(plog[:], cst["rwt"][:, kc, :],
                                     hnT32[:, kc, :],
                                     start=(kc == 0), stop=(kc == KC - 1))
                lsb = work.tile([E, HS], f32r, tag="lsb", name="lsb")
                nc.scalar.copy(lsb[:], plog[:])
                for q in range(2):
                    qg = hs * 2 + q
                    ptr = P((128, E), f32r)
                    nc.tensor.transpose(ptr[:], lsb[:, q * 128:(q + 1) * 128],
                                        cst["ident32"][0:E, 0:E])
                    nt = work.tile([128, E], f32, tag="nt", name="nt")
                    nc.sync.dma_start(nt[:], io["nsh"][qg * 128:(qg + 1) * 128, :])
                    zt = work.tile([128, E], f32, tag="zt", name="zt")
                    nc.vector.tensor_tensor(zt[:], ptr[:], nt[:], op=OP.add)
                    ez = work.tile([128, E], f32, tag="ez", name="ez")
                    den = work.tile([128, 1], f32, tag="den", name="den")
                    nc.scalar.activation(ez[:], zt[:], AF.Exp,
                                         accum_out=den[:])
                    rd = work.tile([128, 1], f32, tag="den", name="rd")
                    nc.vector.reciprocal(rd[:], den[:])
                    pr = work.tile([128, E], f32, tag="pr", name="pr")
                    nc.vector.tensor_scalar_mul(pr[:], ez[:], rd[:])
                    m1 = work.tile([128, 1], f32, tag="m1", name="m1")
                    nc.vector.reduce_max(m1[:], pr[:], axis=AX.X)
                    eqm = work.tile([128, E], f32, tag="eqm", name="eqm")
                    nc.vector.tensor_scalar(eqm[:], pr[:], m1[:], None,
                                            op0=OP.is_ge)
                    msk = work.tile([128, E], f32, tag="msk", name="msk")
                    nc.vector.scalar_tensor_tensor(msk[:], eqm[:], -30000.0,
                                                   pr[:], op0=OP.mult,
                                                   op1=OP.add)
                    m2 = work.tile([128, 1], f32, tag="m1", name="m2")
                    nc.vector.reduce_max(m2[:], msk[:], axis=AX.X)
                    ind = work.tile([128, E], f32, tag="ind", name="ind")
                    nc.vector.tensor_scalar(ind[:], pr[:], m2[:], None,
                                            op0=OP.is_ge)
                    nc.vector.tensor_tensor(wgt[:, qg * E:(qg + 1) * E],
                                            pr[:], ind[:], op=OP.mult)

            # expert FFN sweep over the shard
            for e in range(E):
                if e >= 2:
                    load_expert(e)
                w1e, w2e, woe = wt[e]
                for hs in range(2):
                    peo = [P() for _ in range(4)]  # (q, db) output accumulators
                    # wout matmuls for hidden chunk hc issue one step behind
                    # the h-chunk production so PE never waits on silu/htc.
                    htcs = {}
                    for hc in range(KC + 1):
                        if hc < KC:
                            p1 = P((128, HS))
                            for kc in range(KC):
                                nc.tensor.matmul(
                                    p1[:], w1e[:, kc, hc * 128:(hc + 1) * 128],
                                    hnT[hs][:, kc, :],
                                    start=(kc == 0), stop=(kc == KC - 1))
                            p2 = P((128, HS))
                            for kc in range(KC):
                                nc.tensor.matmul(
                                    p2[:], w2e[:, kc, hc * 128:(hc + 1) * 128],
                                    hnT[hs][:, kc, :],
                                    start=(kc == 0), stop=(kc == KC - 1))
                            s1 = work.tile([128, HS], bf16, tag="s1", name="s1")
                            nc.scalar.activation(
                                s1[:], p1[:], AF.Silu,
                                bias=cst["b1h"][:, e * KC + hc:e * KC + hc + 1],
                                scale=1.0)
                            htc = work.tile([128, HS], bf16, tag="htc",
                                            name="htc")
                            nc.vector.scalar_tensor_tensor(
                                htc[:], p2[:],
                                cst["b2h"][:, e * KC + hc:e * KC + hc + 1],
                                s1[:], op0=OP.add, op1=OP.mult)
                            htcs[hc] = htc
                        ha = hc - 1
                        if ha >= 0:
                            for q in range(2):
                                for db in range(2):
                                    nc.tensor.matmul(
                                        peo[q * 2 + db][:],
                                        htcs[ha][:, q * 128:(q + 1) * 128],
                                        woe[:, ha, db * 512:(db + 1) * 512],
                                        start=(ha == 0), stop=(ha == KC - 1))
                            del htcs[ha]
                    for q in range(2):
                        qg = hs * 2 + q
                        for db in range(2):
                            nc.vector.scalar_tensor_tensor(
                                acc[qg][:, db * 512:(db + 1) * 512],
                                peo[q * 2 + db][:],
                                wgt[:, qg * E + e:qg * E + e + 1],
                                acc[qg][:, db * 512:(db + 1) * 512],
                                op0=OP.mult, op1=OP.add)
            for qg in range(4):
                nc.sync.dma_start(io["out"][qg * 128:(qg + 1) * 128, :],
                                  acc[qg][:])

    nc.compile()
    return nc


# =====================================================================
# Host-side input prep / output combine
# =====================================================================
def prep_in_maps(inputs):
    np_bf16 = mybir.dt.np(bf16)
    x = np.asarray(inputs["x"], np.float32).reshape(T, D)
    scale1 = np.asarray(inputs["scale1"], np.float32)
    scale2 = np.asarray(inputs["scale2"], np.float32)
    wq = scale1[:, None] * np.asarray(inputs["wq"], np.float32)
    wk = scale1[:, None] * np.asarray(inputs["wk"], np.float32)
    wv = scale1[:, None] * np.asarray(inputs["wv"], np.float32)
    wo = np.asarray(inputs["wo"], np.float32)
    rw = scale2[:, None] * np.asarray(inputs["router_w"], np.float32)
    w1 = scale2[None, :, None] * np.asarray(inputs["w1"], np.float32)
    w2 = scale2[None, :, None] * np.asarray(inputs["w2"], np.float32)
    wout = np.asarray(inputs["wout"], np.float32)
    b1 = np.asarray(inputs["b1"], np.float32)
    b2 = np.asarray(inputs["b2"], np.float32)

    import jax
    noise = np.asarray(jax.random.gumbel(jax.random.key(42), (B, S, E),
                                         np.float32)) * 0.05
    noise = noise.reshape(T, E).astype(np.float32)

    half = DK // 2
    inv = 1.0 / (10000.0 ** (np.arange(half, dtype=np.float32) / half))
    ang = np.arange(S, dtype=np.float32)[:, None] * inv[None, :]  # [S, 32]
    cos_h = np.cos(ang).T  # [32, S]
    sin_h = np.sin(ang).T
    blk_cos = np.concatenate([cos_h, cos_h], 0)        # [64, S]
    blk_sin = np.concatenate([sin_h, sin_h], 0)
    cosb = np.concatenate([blk_cos, blk_cos], 0).astype(np_bf16)  # [128, S]
    sinb = np.concatenate([blk_sin, blk_sin], 0).astype(np_bf16)

    masks01 = np.zeros((128, 4, 512), np.float32)
    kr = np.arange(128)[:, None]
    qc = np.arange(512)[None, :]
    for t in range(4):
        masks01[:, t, :] = np.where(kr + 128 * t <= qc, 1.0, 0.0)
    masks01 = masks01.astype(np_bf16)

    identb = np.eye(128, dtype=np.float32).astype(np_bf16)
    id64b = np.zeros((128, 128), np.float32)
    id64b[64:128, 0:64] = np.eye(64, dtype=np.float32)
    id64b = id64b.astype(np_bf16)
    ident32 = np.eye(128, dtype=np.float32)

    def chunk_rows(w):  # [D, M] -> [128, KC, M] with rows = kc*128 + p
        return np.ascontiguousarray(
            w.reshape(KC, 128, w.shape[1]).transpose(1, 0, 2))

    rwt = chunk_rows(rw)                                 # [128, KC, E] fp32
    wo_t = np.ascontiguousarray(
        wo.reshape(NC, 128, D).transpose(1, 0, 2)).astype(np_bf16)
    w1t = np.stack([chunk_rows(w1[e]).astype(np_bf16) for e in range(E)], 0)
    w2t = np.stack([chunk_rows(w2[e]).astype(np_bf16) for e in range(E)], 0)
    woutt = np.stack([chunk_rows(wout[e]).astype(np_bf16) for e in range(E)], 0)
    b1h = np.concatenate([b1[e].reshape(KC, 128).T for e in range(E)],
                         1).astype(np.float32)           # [128, E*KC]
    b2h = np.concatenate([b2[e].reshape(KC, 128).T for e in range(E)],
                         1).astype(np.float32)

    def swap(w):
        ws = np.empty_like(w)
        for hh in range(2):
            r = hh * 64
            ws[:, r:r + 32] = -w[:, r + 32:r + 64]
            ws[:, r + 32:r + 64] = w[:, r:r + 32]
        return ws

    in_maps = []
    for c in range(NC):
        cols = slice(c * 128, (c + 1) * 128)
        wq_c = np.ascontiguousarray(wq[:, cols])
        wk_c = np.ascontiguousarray(wk[:, cols])
        wv_c = np.ascontiguousarray(wv[:, cols])
        in_maps.append({
            "x": x,
            "xsh": np.ascontiguousarray(x[c * SH:(c + 1) * SH]),
            "nsh": np.ascontiguousarray(noise[c * SH:(c + 1) * SH]),
            "wq": chunk_rows(wq_c).astype(np_bf16),
            "wk": chunk_rows(wk_c).astype(np_bf16),
            "wv": chunk_rows(wv_c).astype(np_bf16),
            "wqs": chunk_rows(swap(wq_c)).astype(np_bf16),
            "wks": chunk_rows(swap(wk_c)).astype(np_bf16),
            "wo": wo_t,
            "cosb": cosb, "sinb": sinb, "masks01": masks01,
            "identb": identb, "id64b": id64b, "ident32": ident32,
            "rwt": rwt,
            "w1t": w1t, "w2t": w2t, "woutt": woutt,
            "b1h": b1h, "b2h": b2h,
        })
    return in_maps


def combine(results):
    y = np.concatenate([results[c]["out"] for c in range(NC)], axis=0)
    return np.ascontiguousarray(y.astype(np.float32).reshape(B, S, D))


# ---------------------------------------------------------------------
# PJRT runner (axon): persistent jitted executable for the SPMD launch.
# ---------------------------------------------------------------------
import jax
from jax.sharding import Mesh, PartitionSpec
from jax.experimental.shard_map import shard_map
from concourse import bass2jax


def make_runner(nc, n_cores):
    bass2jax.install_neuronx_cc_hook()
    partition_name = nc.partition_id_tensor.name if nc.partition_id_tensor else None
    in_names, out_names, out_avals, zero_outs = [], [], [], []
    for alloc in nc.m.functions[0].allocations:
        if not isinstance(alloc, mybir.MemoryLocationSet):
            continue
        name = alloc.memorylocations[0].name
        if alloc.kind == "ExternalInput":
            if name != partition_name:
                in_names.append(name)
        elif alloc.kind == "ExternalOutput":
            out_names.append(name)
            shape = tuple(alloc.tensor_shape)
            dtype = mybir.dt.np(alloc.dtype)
            out_avals.append(jax.core.ShapedArray(shape, dtype))
            zero_outs.append(np.zeros(shape, dtype))
    n_params = len(in_names)
    n_outs = len(out_avals)
    all_in_names = list(in_names) + list(out_names)
    if partition_name is not None:
        all_in_names.append(partition_name)

    def _body(*args):
        operands = list(args)
        if partition_name is not None:
            operands.append(bass2jax.partition_id_tensor())
        outs = bass2jax._bass_exec_p.bind(
            *operands,
            out_avals=tuple(out_avals),
            in_names=tuple(all_in_names),
            out_names=tuple(out_names),
            lowering_input_output_aliases=(),
            sim_require_finite=True,
            sim_require_nnan=True,
            nc=nc,
        )
        return tuple(outs)

    devices = jax.devices()[:n_cores]
    mesh = Mesh(np.asarray(devices), ("core",))
    in_specs = (PartitionSpec("core"),) * (n_params + n_outs)
    out_specs = (PartitionSpec("core"),) * n_outs
    donate = tuple(range(n_params, n_params + n_outs))
    sharded = jax.jit(
        shard_map(_body, mesh=mesh, in_specs=in_specs, out_specs=out_specs,
                  check_rep=False),
        donate_argnums=donate, keep_unused=True,
    )

    def run(in_maps):
        per_core = [[np.asarray(m[name]) for name in in_names] for m in in_maps]
        concat_in = [np.concatenate([per_core[c][i] for c in range(n_cores)], axis=0)
                     for i in range(n_params)]
        concat_zeros = [np.zeros((n_cores * z.shape[0], *z.shape[1:]), z.dtype)
                        for z in zero_outs]
        out_arrs = sharded(*concat_in, *concat_zeros)
        out_arrs = [np.asarray(o) for o in out_arrs]
        return [
            {name: out_arrs[i].reshape(n_cores, *out_avals[i].shape)[c]
             for i, name in enumerate(out_names)}
            for c in range(n_cores)
        ]

    return run


_CACHE = {}


def kernel(**inputs):
    if "nc" not in _CACHE:
        _CACHE["nc"] = build_program()
        _CACHE["run"] = make_runner(_CACHE["nc"], NC)
    in_maps = prep_in_maps(inputs)
    results = _CACHE["run"](in_maps)
    return combine(results)


# revision 33
# speedup vs baseline: 2.2105x; 1.0576x over previous
"""Self-contained Trainium2 Bass kernel for nn_DariushLayer_14087492731059.

kernel(**inputs) takes the FULL unsharded inputs of reference.setup_inputs()
and returns the full [B, S, D] float32 output, computed across 8 NeuronCores.

Phase A: attention tensor-parallel over heads (2 heads/core), bf16 compute.
Boundary: two bf16 AllToAll collectives exchange per-head attention outputs
so that each core ends up owning a 512-token shard (tokens c*512..(c+1)*512).
Phase B: out-projection + residual + rmsnorm + router + ALL 8 experts run
token-sharded on each core's shard (dense MoE = every expert sees every
token, so token sharding does the same FLOPs as expert sharding but needs no
broadcast of activations).  Expert weights stream from HBM in bf16.
Router runs in fp32r for bit-accurate top-2 selection.
Host side: concatenate the 8 output shards.
"""

import numpy as np
import concourse.bass as bass
import concourse.tile as tile
from concourse import bacc, mybir
from contextlib import ExitStack

f32, f32r, bf16 = mybir.dt.float32, mybir.dt.float32r, mybir.dt.bfloat16
AF = mybir.ActivationFunctionType
OP = mybir.AluOpType
AX = mybir.AxisListType

B, S, D, H, DK, E = 2, 2048, 1024, 16, 64, 8
T = B * S
NC = 8
KC = D // 128
SH = T // NC          # tokens per core shard (512)
HS = SH // 2          # half shard (256)
EPS = 1e-6


def build_program():
    nc = bacc.Bacc("TRN2", target_bir_lowering=False, debug=False, num_devices=NC)
    dt = nc.dram_tensor
    io = {}
    def inp(nm, shp, ty=f32):
        io[nm] = dt(nm, shp, ty, kind="ExternalInput").ap()
    def outp(nm, shp, ty=f32):
        io[nm] = dt(nm, shp, ty, kind="ExternalOutput").ap()

    inp("x", [T, D])                       # full input (phase A norm)
    inp("xsh", [SH, D])                    # this core's token shard of x
    inp("nsh", [SH, E])                    # gumbel noise shard
    for nm in ("wq", "wk", "wv", "wqs", "wks"):
        inp(nm, [128, KC, 128], bf16)      # head-sliced qkv weights [p,kc,m]
    inp("wo", [128, NC, D], bf16)          # full out-proj, src-core-major rows
    inp("cosb", [128, S], bf16); inp("sinb", [128, S], bf16)
    inp("masks01", [128, 4, 512], bf16)    # multiplicative causal masks
    inp("identb", [128, 128], bf16); inp("id64b", [128, 128], bf16)
    inp("ident32", [128, 128], f32r)
    inp("rwt", [128, KC, E], f32r)         # router weights [p,kc,e]
    inp("w1t", [E, 128, KC, D], bf16)      # per-expert gate proj [p,kc,hc*128+m]
    inp("w2t", [E, 128, KC, D], bf16)
    inp("woutt", [E, 128, KC, D], bf16)    # per-expert out proj [p,hc,m]
    inp("b1h", [128, E * KC]); inp("b2h", [128, E * KC])
    outp("out", [SH, D])

    with tile.TileContext(nc) as tc, ExitStack() as top:
        const = top.enter_context(tc.tile_pool(name="const", bufs=1))
        psum = top.enter_context(tc.tile_pool(name="psum", bufs=8, space="PSUM"))
        dram = top.enter_context(tc.tile_pool(name="dram", bufs=1, space="DRAM"))

        # PSUM is 8 banks; every tile takes a full bank, so tag rings must
        # sum to <= 8: "big" (pse/pa/pb/pv/pp/peo) 4, "med" (transposes,
        # p1/p2, router) 2, "pos" (attention accumulators) 2.
        def P(shape=(128, 512), ty=f32, tag="big", bufs=4):
            return psum.tile(list(shape), ty, tag=tag, name=tag, bufs=bufs)

        def Pm(shape, ty=f32):
            return psum.tile(list(shape), ty, tag="med", name="med", bufs=2)

        cst = {}
        for nm, shp, ty in [("identb", [128, 128], bf16),
                            ("id64b", [128, 128], bf16),
                            ("ident32", [128, 128], f32r),
                            ("rwt", [128, KC, E], f32r),
                            ("b1h", [128, E * KC], f32),
                            ("b2h", [128, E * KC], f32)]:
            cst[nm] = const.tile(shp, ty, name=nm)
            nc.sync.dma_start(cst[nm][:], io[nm][:])
        eps_t = const.tile([128, 1], f32, name="eps_t")
        nc.vector.memset(eps_t[:], EPS)

        a2a_in = [dram.tile([NC, 128, HS], bf16, name=f"a2a_in{i}")
                  for i in range(2)]
        a2a_out = [dram.tile([NC, 128, HS], bf16, name=f"a2a_out{i}")
                   for i in range(2)]

        # --- rmsnorm one [128, D] fp32 row-tile already in SBUF -> rr [128,1]
        # (sq_scratch is an overwritten-later [128, D] tile reused as the
        #  Square output buffer)
        def rms_rr(work, xt, sq_scratch):
            ssum = work.tile([128, 1], f32, tag="ssum", name="ssum")
            nc.scalar.activation(sq_scratch[:], xt[:], AF.Square,
                                 accum_out=ssum[:])
            sd = work.tile([128, 1], f32, tag="ssum", name="sd")
            nc.scalar.activation(sd[:], ssum[:], AF.Sqrt, bias=eps_t[:],
                                 scale=1.0 / D)
            rr = work.tile([128, 1], f32, tag="ssum", name="rr")
            nc.vector.reciprocal(rr[:], sd[:])
            return rr

        # =================================================================
        # Phase A: attention (this core's 2 heads, all T tokens)
        # =================================================================
        with tc.tile_pool(name="qkv", bufs=1) as qkv, \
             tc.tile_pool(name="apool", bufs=1) as apool:
            qT = qkv.tile([128, T], bf16, name="qT")
            kT = qkv.tile([128, T], bf16, name="kT")
            vT = qkv.tile([128, T], bf16, name="vT")
            oT = qkv.tile([128, T], bf16, name="oT")
            for nm, shp, ty in [("cosb", [128, S], bf16), ("sinb", [128, S], bf16),
                                ("masks01", [128, 4, 512], bf16)]:
                cst[nm] = apool.tile(shp, ty, name=nm)
                nc.sync.dma_start(cst[nm][:], io[nm][:])
            for nm in ("wq", "wk", "wv", "wqs", "wks"):
                cst[nm] = apool.tile([128, KC, 128], bf16, name=nm)
                nc.sync.dma_start(cst[nm][:], io[nm][:])

            with tc.tile_pool(name="xnt", bufs=3) as xnt_pool, \
                 tc.tile_pool(name="work", bufs=4) as work:
                for b in range(B):
                    for sb in range(4):
                        xnT = xnt_pool.tile([128, KC, 512], bf16, tag="xnT",
                                            name="xnT")
                        for q4 in range(4):
                            st = b * 16 + sb * 4 + q4
                            r0 = st * 128
                            xt = work.tile([128, D], f32, tag="xt", name="xt")
                            nc.sync.dma_start(xt[:], io["x"][r0:r0 + 128, :])
                            xh = work.tile([128, D], bf16, tag="xh", name="xh")
                            rr = rms_rr(work, xt, xh)
                            nc.scalar.mul(xh[:], xt[:], rr[:])
                            for kg in range(2):
                                pt = Pm((128, 512), bf16)
                                for kk in range(4):
                                    kc = kg * 4 + kk
                                    nc.tensor.transpose(
                                        pt[:, kk * 128:(kk + 1) * 128],
                                        xh[:, kc * 128:(kc + 1) * 128],
                                        cst["identb"][:])
                                nc.vector.tensor_copy(
                                    xnT[:, kg * 4:(kg + 1) * 4,
                                        q4 * 128:(q4 + 1) * 128],
                                    pt[:].rearrange("p (k m) -> p k m", k=4))
                        gl = slice(b * S + sb * 512, b * S + (sb + 1) * 512)
                        sl = slice(sb * 512, (sb + 1) * 512)
                        for base, swp, dst in (("wq", "wqs", qT),
                                               ("wk", "wks", kT)):
                            pa = P()
                            for kc in range(KC):
                                nc.tensor.matmul(pa[:], cst[base][:, kc, :],
                                                 xnT[:, kc, :],
                                                 start=(kc == 0),
                                                 stop=(kc == KC - 1))
                            pb = P()
                            for kc in range(KC):
                                nc.tensor.matmul(pb[:], cst[swp][:, kc, :],
                                                 xnT[:, kc, :],
                                                 start=(kc == 0),
                                                 stop=(kc == KC - 1))
                            t1 = work.tile([128, 512], f32, tag="t1", name="t1")
                            nc.vector.tensor_tensor(t1[:], pa[:],
                                                    cst["cosb"][:, sl],
                                                    op=OP.mult)
                            t2 = work.tile([128, 512], f32, tag="t2", name="t2")
                            nc.vector.tensor_tensor(t2[:], pb[:],
                                                    cst["sinb"][:, sl],
                                                    op=OP.mult)
                            nc.gpsimd.tensor_tensor(dst[:, gl], t1[:], t2[:],
                                                    op=OP.add)
                        pv = P()
                        for kc in range(KC):
                            nc.tensor.matmul(pv[:], cst["wv"][:, kc, :],
                                             xnT[:, kc, :],
                                             start=(kc == 0), stop=(kc == KC - 1))
                        nc.scalar.copy(vT[:, gl], pv[:])

            # attention core
            with tc.tile_pool(name="att", bufs=3) as att, \
                 tc.tile_pool(name="expp", bufs=5) as expp, \
                 tc.tile_pool(name="vsb", bufs=18) as vsbp:
                for b in range(B):
                    for h in range(2):
                        hr = slice(h * 64, (h + 1) * 64)
                        idn = cst["identb"] if h == 0 else cst["id64b"]
                        vchunks = []
                        for m in range(16):
                            gk = slice(b * S + m * 128, b * S + (m + 1) * 128)
                            pt = Pm((128, 64), bf16)
                            nc.tensor.transpose(pt[:], vT[hr, gk], idn[hr, 0:64])
                            vs = vsbp.tile([128, 66], bf16, tag="vs", name="vs")
                            nc.vector.tensor_copy(vs[:, 0:64], pt[:])
                            nc.vector.memset(vs[:, 64:65], 1.0)
                            nc.vector.memset(vs[:, 65:66], 0.0)
                            vchunks.append(vs)
                        for jq in range(4):
                            gq = slice(b * S + jq * 512, b * S + (jq + 1) * 512)
                            nch = 4 * jq + 4
                            pos = psum.tile([128, 264], f32, tag="pos",
                                            name="pos", bufs=2)
                            # 3-deep software pipeline: score matmul + exp for
                            # step m+3 issue before the AV matmuls of step m,
                            # so the PE never waits on the Act/DVE chain.
                            # Causal structure per 128x128 sub-block: for the
                            # diagonal key chunk (t = m-4jq) only the qt == t
                            # sub-block needs masking; qt < t sub-blocks are
                            # fully masked and their AV matmuls are skipped.
                            DEPTH = 3
                            ets = {}
                            for m in range(nch + DEPTH):
                                if m < nch:
                                    t = m - 4 * jq  # >=0 on the diagonal chunk
                                    lo = max(t, 0) * 128
                                    gk = slice(b * S + m * 128,
                                               b * S + (m + 1) * 128)
                                    pse = P()
                                    nc.tensor.matmul(pse[:], kT[hr, gk],
                                                     qT[hr, gq],
                                                     start=True, stop=True)
                                    et = expp.tile([128, 512], bf16, tag="et",
                                                   name="et")
                                    nc.scalar.activation(et[:, lo:512],
                                                         pse[:, lo:512],
                                                         AF.Exp, scale=0.125)
                                    if t >= 0:
                                        nc.vector.tensor_tensor(
                                            et[:, lo:lo + 128],
                                            et[:, lo:lo + 128],
                                            cst["masks01"][:, 0, 0:128],
                                            op=OP.mult)
                                    ets[m] = et
                                ma = m - DEPTH
                                if ma >= 0:
                                    ta = ma - 4 * jq
                                    for qt in range(max(ta, 0), 4):
                                        nc.tensor.matmul(
                                            pos[:, qt * 66:(qt + 1) * 66],
                                            ets[ma][:, qt * 128:(qt + 1) * 128],
                                            vchunks[ma][:],
                                            start=(ma == 0),
                                            stop=(ma == 4 * jq + qt))
                                    del ets[ma]
                            for qt in range(4):
                                rcp = att.tile([128, 1], f32, tag="rcp",
                                               name="rcp")
                                nc.vector.reciprocal(
                                    rcp[:], pos[:, qt * 66 + 64:qt * 66 + 65])
                                opr = att.tile([128, 64], bf16, tag="opr",
                                               name="opr")
                                nc.vector.tensor_scalar_mul(
                                    opr[:], pos[:, qt * 66:qt * 66 + 64],
                                    rcp[:])
                                ptt = Pm((128, 128), bf16)
                                nc.tensor.transpose(ptt[0:64, :], opr[:],
                                                    cst["identb"][:])
                                g128 = slice(b * S + jq * 512 + qt * 128,
                                             b * S + jq * 512 + (qt + 1) * 128)
                                nc.vector.tensor_copy(oT[hr, g128],
                                                      ptt[0:64, :])

                # ship per-head attention outputs to token-shard owners
                for j in range(NC):
                    c0 = j * SH
                    nc.sync.dma_start(a2a_in[0][j], oT[:, c0:c0 + HS])
                    nc.sync.dma_start(a2a_in[1][j], oT[:, c0 + HS:c0 + SH])

        for i in range(2):
            nc.gpsimd.collective_compute(
                "AllToAll", OP.bypass, replica_groups=[list(range(NC))],
                ins=[a2a_in[i].opt()], outs=[a2a_out[i].opt()])

        # =================================================================
        # Phase B: out-proj + residual + norm + router + all experts,
        # on this core's 512-token shard.
        # =================================================================
        wpool = top.enter_context(tc.tile_pool(name="wts", bufs=2))
        bper = top.enter_context(tc.tile_pool(name="bper", bufs=1))

        wt = {}
        def load_expert(e):
            tiles = []
            for key in ("w1t", "w2t", "woutt"):
                t = wpool.tile([128, KC, D], bf16, tag=key, name=f"{key}{e}")
                nc.sync.dma_start(t[:], io[key][e])
                tiles.append(t)
            wt[e] = tiles

        # prefetch first two experts; these DMAs don't depend on the
        # collectives, so they stream during the AllToAll wait
        load_expert(0)
        load_expert(1)

        with tc.tile_pool(name="workb", bufs=3) as work:
            wo_t = bper.tile([128, NC, D], bf16, name="wo_t")
            nc.sync.dma_start(wo_t[:], io["wo"][:])
            acc = [bper.tile([128, D], f32, name=f"acc{q}") for q in range(4)]
            hnT = [bper.tile([128, KC, HS], bf16, name=f"hnT{i}")
                   for i in range(2)]
            wgt = bper.tile([128, 4 * E], f32, name="wgt")
            aot = [bper.tile([128, NC, HS], bf16, name=f"aot{i}")
                   for i in range(2)]
            for hs in range(2):
                nc.sync.dma_start(aot[hs][:],
                                  a2a_out[hs].rearrange("n p m -> p n m"))
                hnT32 = work.tile([128, KC, HS], f32r, tag="hnT32",
                                  name="hnT32", bufs=2)
                for q in range(2):
                    qg = hs * 2 + q
                    r0 = qg * 128
                    xst = work.tile([128, D], f32, tag="xst", name="xst",
                                    bufs=2)
                    nc.sync.dma_start(xst[:], io["xsh"][r0:r0 + 128, :])
                    for db in range(2):
                        pp = P()
                        for src in range(NC):
                            nc.tensor.matmul(
                                pp[:], aot[hs][:, src, q * 128:(q + 1) * 128],
                                wo_t[:, src, db * 512:(db + 1) * 512],
                                start=(src == 0), stop=(src == NC - 1))
                        nc.vector.tensor_tensor(
                            acc[qg][:, db * 512:(db + 1) * 512],
                            xst[:, db * 512:(db + 1) * 512], pp[:], op=OP.add)
                    xh32 = work.tile([128, D], f32r, tag="xh32", name="xh32",
                                     bufs=2)
                    rr = rms_rr(work, acc[qg], xh32)
                    nc.vector.tensor_scalar_mul(xh32[:], acc[qg][:], rr[:])
                    for kg in range(2):
                        pt32 = Pm((128, 512), f32r)
                        for kk in range(4):
                            kc = kg * 4 + kk
                            nc.tensor.transpose(
                                pt32[:, kk * 128:(kk + 1) * 128],
                                xh32[:, kc * 128:(kc + 1) * 128],
                                cst["ident32"][:])
                        nc.vector.tensor_copy(
                            hnT32[:, kg * 4:(kg + 1) * 4,
                                  q * 128:(q + 1) * 128],
                            pt32[:].rearrange("p (k m) -> p k m", k=4))
                    nc.scalar.copy(hnT[hs][:, :, q * 128:(q + 1) * 128],
                                   hnT32[:, :, q * 128:(q + 1) * 128])
                # router (fp32r for exact top-2)
                plog = Pm((E, HS))
                for kc in range(KC):
                    nc.tensor.matmul(plog[:], cst["rwt"][:, kc, :],
                                     hnT32[:, kc, :],
                                     start=(kc == 0), stop=(kc == KC - 1))
                lsb = work.tile([E, HS], f32r, tag="lsb", name="lsb")
                nc.scalar.copy(lsb[:], plog[:])
                for q in range(2):
                    qg = hs * 2 + q
                    ptr = Pm((128, E), f32r)
                    nc.tensor.transpose(ptr[:], lsb[:, q * 128:(q + 1) * 128],
                                        cst["ident32"][0:E, 0:E])
                    nt = work.tile([128, E], f32, tag="nt", name="nt")
                    nc.sync.dma_start(nt[:], io["nsh"][qg * 128:(qg + 1) * 128, :])
                    zt = work.tile([128, E], f32, tag="zt", name="zt")
                    nc.vector.tensor_tensor(zt[:], ptr[:], nt[:], op=OP.add)
                    ez = work.tile([128, E], f32, tag="ez", name="ez")
                    den = work.tile([128, 1], f32, tag="den", name="den")
                    nc.scalar.activation(ez[:], zt[:], AF.Exp,
                                         accum_out=den[:])
                    rd = work.tile([128, 1], f32, tag="den", name="rd")
                    nc.vector.reciprocal(rd[:], den[:])
                    pr = work.tile([128, E], f32, tag="pr", name="pr")
                    nc.vector.tensor_scalar_mul(pr[:], ez[:], rd[:])
                    m1 = work.tile([128, 1], f32, tag="m1", name="m1")
                    nc.vector.reduce_max(m1[:], pr[:], axis=AX.X)
                    eqm = work.tile([128, E], f32, tag="eqm", name="eqm")
                    nc.vector.tensor_scalar(eqm[:], pr[:], m1[:], None,
                                            op0=OP.is_ge)
                    msk = work.tile([128, E], f32, tag="msk", name="msk")
                    nc.vector.scalar_tensor_tensor(msk[:], eqm[:], -30000.0,
                                                   pr[:], op0=OP.mult,
                                                   op1=OP.add)
                    m2 = work.tile([128, 1], f32, tag="m1", name="m2")
                    nc.vector.reduce_max(m2[:], msk[:], axis=AX.X)
                    ind = work.tile([128, E], f32, tag="ind", name="ind")
                    nc.vector.tensor_scalar(ind[:], pr[:], m2[:], None,
                                            op0=OP.is_ge)
                    nc.vector.tensor_tensor(wgt[:, qg * E:(qg + 1) * E],
                                            pr[:], ind[:], op=OP.mult)

            # expert FFN sweep over the shard
            for e in range(E):
                if e >= 2:
                    load_expert(e)
                w1e, w2e, woe = wt[e]
                for hs in range(2):
                    peo = [P() for _ in range(4)]  # (q, db) output accumulators
                    # wout matmuls for hidden chunk hc issue one step behind
                    # the h-chunk production so PE never waits on silu/htc.
                    htcs = {}
                    for hc in range(KC + 1):
                        if hc < KC:
                            p1 = Pm((128, HS))
                            for kc in range(KC):
                                nc.tensor.matmul(
                                    p1[:], w1e[:, kc, hc * 128:(hc + 1) * 128],
                                    hnT[hs][:, kc, :],
                                    start=(kc == 0), stop=(kc == KC - 1))
                            p2 = Pm((128, HS))
                            for kc in range(KC):
                                nc.tensor.matmul(
                                    p2[:], w2e[:, kc, hc * 128:(hc + 1) * 128],
                                    hnT[hs][:, kc, :],
                                    start=(kc == 0), stop=(kc == KC - 1))
                            s1 = work.tile([128, HS], bf16, tag="s1", name="s1")
                            nc.scalar.activation(
                                s1[:], p1[:], AF.Silu,
                                bias=cst["b1h"][:, e * KC + hc:e * KC + hc + 1],
                                scale=1.0)
                            htc = work.tile([128, HS], bf16, tag="htc",
                                            name="htc")
                            nc.vector.scalar_tensor_tensor(
                                htc[:], p2[:],
                                cst["b2h"][:, e * KC + hc:e * KC + hc + 1],
                                s1[:], op0=OP.add, op1=OP.mult)
                            htcs[hc] = htc
                        ha = hc - 1
                        if ha >= 0:
                            for q in range(2):
                                for db in range(2):
                                    nc.tensor.matmul(
                                        peo[q * 2 + db][:],
                                        htcs[ha][:, q * 128:(q + 1) * 128],
                                        woe[:, ha, db * 512:(db + 1) * 512],
                                        start=(ha == 0), stop=(ha == KC - 1))
                            del htcs[ha]
                    for q in range(2):
                        qg = hs * 2 + q
                        for db in range(2):
                            nc.vector.scalar_tensor_tensor(
                                acc[qg][:, db * 512:(db + 1) * 512],
                                peo[q * 2 + db][:],
                                wgt[:, qg * E + e:qg * E + e + 1],
                                acc[qg][:, db * 512:(db + 1) * 512],
                                op0=OP.mult, op1=OP.add)
            for qg in range(4):
                nc.sync.dma_start(io["out"][qg * 128:(qg + 1) * 128, :],
                                  acc[qg][:])

    nc.compile()
    return nc


# =====================================================================
# Host-side input prep / output combine
# =====================================================================
def prep_in_maps(inputs):
    np_bf16 = mybir.dt.np(bf16)
    x = np.asarray(inputs["x"], np.float32).reshape(T, D)
    scale1 = np.asarray(inputs["scale1"], np.float32)
    scale2 = np.asarray(inputs["scale2"], np.float32)
    wq = scale1[:, None] * np.asarray(inputs["wq"], np.float32)
    wk = scale1[:, None] * np.asarray(inputs["wk"], np.float32)
    wv = scale1[:, None] * np.asarray(inputs["wv"], np.float32)
    wo = np.asarray(inputs["wo"], np.float32)
    rw = scale2[:, None] * np.asarray(inputs["router_w"], np.float32)
    w1 = scale2[None, :, None] * np.asarray(inputs["w1"], np.float32)
    w2 = scale2[None, :, None] * np.asarray(inputs["w2"], np.float32)
    wout = np.asarray(inputs["wout"], np.float32)
    b1 = np.asarray(inputs["b1"], np.float32)
    b2 = np.asarray(inputs["b2"], np.float32)

    import jax
    noise = np.asarray(jax.random.gumbel(jax.random.key(42), (B, S, E),
                                         np.float32)) * 0.05
    noise = noise.reshape(T, E).astype(np.float32)

    half = DK // 2
    inv = 1.0 / (10000.0 ** (np.arange(half, dtype=np.float32) / half))
    ang = np.arange(S, dtype=np.float32)[:, None] * inv[None, :]  # [S, 32]
    cos_h = np.cos(ang).T  # [32, S]
    sin_h = np.sin(ang).T
    blk_cos = np.concatenate([cos_h, cos_h], 0)        # [64, S]
    blk_sin = np.concatenate([sin_h, sin_h], 0)
    cosb = np.concatenate([blk_cos, blk_cos], 0).astype(np_bf16)  # [128, S]
    sinb = np.concatenate([blk_sin, blk_sin], 0).astype(np_bf16)

    masks01 = np.zeros((128, 4, 512), np.float32)
    kr = np.arange(128)[:, None]
    qc = np.arange(512)[None, :]
    for t in range(4):
        masks01[:, t, :] = np.where(kr + 128 * t <= qc, 1.0, 0.0)
    masks01 = masks01.astype(np_bf16)

    identb = np.eye(128, dtype=np.float32).astype(np_bf16)
    id64b = np.zeros((128, 128), np.float32)
    id64b[64:128, 0:64] = np.eye(64, dtype=np.float32)
    id64b = id64b.astype(np_bf16)
    ident32 = np.eye(128, dtype=np.float32)

    def chunk_rows(w):  # [D, M] -> [128, KC, M] with rows = kc*128 + p
        return np.ascontiguousarray(
            w.reshape(KC, 128, w.shape[1]).transpose(1, 0, 2))

    rwt = chunk_rows(rw)                                 # [128, KC, E] fp32
    wo_t = np.ascontiguousarray(
        wo.reshape(NC, 128, D).transpose(1, 0, 2)).astype(np_bf16)
    w1t = np.stack([chunk_rows(w1[e]).astype(np_bf16) for e in range(E)], 0)
    w2t = np.stack([chunk_rows(w2[e]).astype(np_bf16) for e in range(E)], 0)
    woutt = np.stack([chunk_rows(wout[e]).astype(np_bf16) for e in range(E)], 0)
    b1h = np.concatenate([b1[e].reshape(KC, 128).T for e in range(E)],
                         1).astype(np.float32)           # [128, E*KC]
    b2h = np.concatenate([b2[e].reshape(KC, 128).T for e in range(E)],
                         1).astype(np.float32)

    def swap(w):
        ws = np.empty_like(w)
        for hh in range(2):
            r = hh * 64
            ws[:, r:r + 32] = -w[:, r + 32:r + 64]
            ws[:, r + 32:r + 64] = w[:, r:r + 32]
        return ws

    in_maps = []
    for c in range(NC):
        cols = slice(c * 128, (c + 1) * 128)
        wq_c = np.ascontiguousarray(wq[:, cols])
        wk_c = np.ascontiguousarray(wk[:, cols])
        wv_c = np.ascontiguousarray(wv[:, cols])
        in_maps.append({
            "x": x,
            "xsh": np.ascontiguousarray(x[c * SH:(c + 1) * SH]),
            "nsh": np.ascontiguousarray(noise[c * SH:(c + 1) * SH]),
            "wq": chunk_rows(wq_c).astype(np_bf16),
            "wk": chunk_rows(wk_c).astype(np_bf16),
            "wv": chunk_rows(wv_c).astype(np_bf16),
            "wqs": chunk_rows(swap(wq_c)).astype(np_bf16),
            "wks": chunk_rows(swap(wk_c)).astype(np_bf16),
            "wo": wo_t,
            "cosb": cosb, "sinb": sinb, "masks01": masks01,
            "identb": identb, "id64b": id64b, "ident32": ident32,
            "rwt": rwt,
            "w1t": w1t, "w2t": w2t, "woutt": woutt,
            "b1h": b1h, "b2h": b2h,
        })
    return in_maps


def combine(results):
    y = np.concatenate([results[c]["out"] for c in range(NC)], axis=0)
    return np.ascontiguousarray(y.astype(np.float32).reshape(B, S, D))


# ---------------------------------------------------------------------
# PJRT runner (axon): persistent jitted executable for the SPMD launch.
# ---------------------------------------------------------------------
import jax
from jax.sharding import Mesh, PartitionSpec
from jax.experimental.shard_map import shard_map
from concourse import bass2jax


def make_runner(nc, n_cores):
    bass2jax.install_neuronx_cc_hook()
    partition_name = nc.partition_id_tensor.name if nc.partition_id_tensor else None
    in_names, out_names, out_avals, zero_outs = [], [], [], []
    for alloc in nc.m.functions[0].allocations:
        if not isinstance(alloc, mybir.MemoryLocationSet):
            continue
        name = alloc.memorylocations[0].name
        if alloc.kind == "ExternalInput":
            if name != partition_name:
                in_names.append(name)
        elif alloc.kind == "ExternalOutput":
            out_names.append(name)
            shape = tuple(alloc.tensor_shape)
            dtype = mybir.dt.np(alloc.dtype)
            out_avals.append(jax.core.ShapedArray(shape, dtype))
            zero_outs.append(np.zeros(shape, dtype))
    n_params = len(in_names)
    n_outs = len(out_avals)
    all_in_names = list(in_names) + list(out_names)
    if partition_name is not None:
        all_in_names.append(partition_name)

    def _body(*args):
        operands = list(args)
        if partition_name is not None:
            operands.append(bass2jax.partition_id_tensor())
        outs = bass2jax._bass_exec_p.bind(
            *operands,
            out_avals=tuple(out_avals),
            in_names=tuple(all_in_names),
            out_names=tuple(out_names),
            lowering_input_output_aliases=(),
            sim_require_finite=True,
            sim_require_nnan=True,
            nc=nc,
        )
        return tuple(outs)

    devices = jax.devices()[:n_cores]
    mesh = Mesh(np.asarray(devices), ("core",))
    in_specs = (PartitionSpec("core"),) * (n_params + n_outs)
    out_specs = (PartitionSpec("core"),) * n_outs
    donate = tuple(range(n_params, n_params + n_outs))
    sharded = jax.jit(
        shard_map(_body, mesh=mesh, in_specs=in_specs, out_specs=out_specs,
                  check_rep=False),
        donate_argnums=donate, keep_unused=True,
    )

    def run(in_maps):
        per_core = [[np.asarray(m[name]) for name in in_names] for m in in_maps]
        concat_in = [np.concatenate([per_core[c][i] for c in range(n_cores)], axis=0)
                     for i in range(n_params)]
        concat_zeros = [np.zeros((n_cores * z.shape[0], *z.shape[1:]), z.dtype)
                        for z in zero_outs]
        out_arrs = sharded(*concat_in, *concat_zeros)
        out_arrs = [np.asarray(o) for o in out_arrs]
        return [
            {name: out_arrs[i].reshape(n_cores, *out_avals[i].shape)[c]
             for i, name in enumerate(out_names)}
            for c in range(n_cores)
        ]

    return run


_CACHE = {}


def kernel(**inputs):
    if "nc" not in _CACHE:
        _CACHE["nc"] = build_program()
        _CACHE["run"] = make_runner(_CACHE["nc"], NC)
    in_maps = prep_in_maps(inputs)
    results = _CACHE["run"](in_maps)
    return combine(results)
